# revision 42
# baseline (speedup 1.0000x reference)
"""Trainium2 Bass kernel for ConditionalAttentionConv2D.

Reference computation (per image, B=8 images total):
    k = maxpool2x2(x @ Wk + bk)          [2304, 16]
    q = x @ Wq + bq                      [9216, 16]
    s = softmax(q @ k^T, axis=-1)        [9216, 2304]
    v = maxpool2x2(x @ Wv + bv)          [2304, 128]
    out = x + beta * (s @ v)             [9216, 128]

Sharding: data-parallel over batch, one image per NeuronCore (8 cores).

Key performance structure:
  - The attention loop is software-pipelined with a 2-half lookahead
    (scores run two half-tiles ahead of the PV matmuls) to minimize PE
    dependency stalls (the HAM clock gate throttles the PE to 1.2 GHz
    whenever its activity window sees idle gaps).
  - exp is split across two engines so it keeps pace with the PE: ACT
    computes exact exp on slots 0-4 (with scale=1/C1), DVE computes a
    Schraudolph-style exp on slots 5-8: scores arrive pre-scaled by
    C1 = 2^7*log2(e) (folded into Wq on the host), DVE adds the exponent
    bias C2 and converts fp32->int16, whose bits ARE the bf16 exp value.
    The softmax denominator (ones column appended to V) sums the actual
    e values used, so rows still normalize exactly.  The first halves are
    ACT-only while DVE drains the preamble maxpool reductions.
  - The first three score tiles of each half run as a 3-way concurrent
    row-tiled group (bands 0/64/32, PSUM banks 0/1/2); the rest as 2-way
    pairs.  k/q are replicated on partitions 0-15, 32-47, 64-79.
  - bk is never applied (softmax is invariant to the per-row constant
    q.bk); beta*bv is folded into the residual input host-side.
  - finish: ACT evacuates o from PSUM, DVE normalizes from SBUF, GpSimd
    adds the residual -- no DVE op ever waits on PV completion, which
    would head-of-line-block the next half's exp in the DVE FIFO.
  - Walrus's LDW optimization is enabled by re-merging tile_legalize's
    standalone Ldweights into their Matmults at the BIR-JSON level (see
    _merge_ldweights).
  - Preamble: xT is transposed on the HOST and loaded by plain DMA
    chunks (Tile serializes every xbar-transpose DMA against all other
    in-flight DMAs -- the HW-deadlock guard -- so transposed loads
    stretched across ~40us of the preamble); pooled-V tiles are
    transposed into vaug by the DMA engine (slot stride 160: the XBAR
    transpose DMA corrupts non-32-aligned destination offsets).
"""

import os
import numpy as np

import ml_dtypes

from concourse import bass, mybir, masks
from concourse.tile import TileContext
from concourse.bass_utils import run_bass_kernel_spmd

# ---------------------------------------------------------------------------
# Walrus in this toolchain rejects >1 sync-wait on a CTRL instruction, but
# TileContext's final drain carries one wait per active proc.  Split them
# across standalone sync-engine NOPs before a bare drain.
# ---------------------------------------------------------------------------


def _patched_drain_and_barrier(self, tick_clock, wait_clock):
    from concourse.vector_clock import ScopedClock

    nop_inst = self.nc.sync.nop(nofuse=True)
    wait_clock.add_sem_waits(
        nop_inst.ins, ScopedClock({None: tick_clock.global_clock})
    )
    si = nop_inst.ins.sync_info
    waits = list(si.on_wait) if si is not None else []
    if len(waits) > 1:
        del si.on_wait[1:]
        for w in waits[1:]:
            n2 = self.nc.sync.nop(nofuse=True)
            n2.ins.sync_info = mybir.SyncInfo(on_wait=[w], on_update=[])
    self.nc.sync.drain()
    self.nc.all_engine_barrier()
    popped = self.nc._tile_sem_poison_stack.pop()
    assert popped is self._sem_poison
    self.nc.clear_and_free_semaphores(list(self.sems.allocated().values()))
    self.nc.all_engine_barrier()


TileContext._drain_and_barrier = _patched_drain_and_barrier


def _tile_structural_classes():
    import concourse.tile as _t

    names = (
        "BassTileCriticalSection", "BassTileConditionalBlock",
        "TileBranchInst", "BassTileRelease",
        "BassTileBranchHintPlaceholder", "BassTileLoopBlock",
    )
    return tuple(getattr(_t, n) for n in names if hasattr(_t, n))


_STRUCTURAL = None
_orig_commit_and_lower = TileContext._commit_and_lower


def _patched_commit_and_lower(self, inst, original_block, old_bb_map,
                              bb_to_exit_bb):
    global _STRUCTURAL
    if _STRUCTURAL is None:
        _STRUCTURAL = _tile_structural_classes()
    si = getattr(inst, "sync_info", None)
    # Ldweights cannot carry waits under walrus's LDW optimization, and no
    # instruction can carry more than one wait on this toolchain: hoist the
    # excess onto same-engine NOPs committed immediately before.
    keep = 0 if isinstance(inst, mybir.InstLdweights) else 1
    if (
        si is not None
        and si.on_wait
        and len(si.on_wait) > keep
        and not isinstance(inst, _STRUCTURAL)
    ):
        waits = list(si.on_wait)
        si.on_wait[:] = waits[len(waits) - keep:]
        for i, w in enumerate(waits[:len(waits) - keep]):
            nop = mybir.InstNoOp(
                name=f"{inst.name}-sw{i}",
                engine=inst.engine,
                sync_info=mybir.SyncInfo(on_wait=[w], on_update=[]),
                bass_nofuse=True,
            )
            self._commit_instruction(nop)
    return _orig_commit_and_lower(
        self, inst, original_block, old_bb_map, bb_to_exit_bb
    )


TileContext._commit_and_lower = _patched_commit_and_lower

# ---------------------------------------------------------------------------
# Enable walrus's LDW optimization (incl. Fast Weight Load: 2x bf16
# weight-load bandwidth).  tile_legalize always splits InstMatmult into a
# standalone InstLdweights + a non-self-loading InstMatmult, which walrus
# rejects under --enable-ldw-opt.  So just before walrus runs, merge each
# Ldweights back into its Matmult (ldweights=true) at the BIR-JSON level
# and flip the flag.  The TileContext commit patch above already keeps
# sem-waits OFF the Ldweights instructions (hoisted to NOPs), so the merge
# only has to migrate on_update entries.
# ---------------------------------------------------------------------------

import json as _json

from concourse import bass_utils as _bu
from concourse import hw_specs as _hw

# ---------------------------------------------------------------------------
# Make the Tile scheduler's cost model see the PE at its HAM-throttled
# 1.2 GHz rate.  The default 2.4 GHz model makes the simulated PV matmuls
# finish early, so the scheduler places PV-dependent ops (o-evacuation,
# normalize) ahead of the next half's exp in the ACT/DVE FIFOs; on real
# (cold) hardware those ops then wait at the FIFO head and stall the PE.
# This only changes instruction scheduling, not hardware behavior.
# ---------------------------------------------------------------------------
_hw.TRN2Spec.PE_CYCLE = 1e9 / 1.2e9


def _merge_ldweights(bir_json: bytes) -> bytes:
    bir = _json.loads(bir_json)
    for fn in bir["functions"]:
        for blk in fn["blocks"]:
            out = []
            pending = None
            for inst in blk["instructions"]:
                op = inst.get("opcode")
                if op == "Ldweights":
                    assert pending is None, "two Ldweights without Matmult"
                    si = inst.get("sync_info") or {}
                    assert not si.get("on_wait"), (
                        f"LDW {inst['name']} carries waits"
                    )
                    pending = inst
                    continue
                if op == "Matmult" and pending is not None:
                    w = pending["ins"][0]
                    mw = inst["ins"][1]
                    assert (
                        w["memref"] == mw["memref"]
                        and w["offset"] == mw["offset"]
                    ), f"LDW/MM pairing mismatch {pending['name']} {inst['name']}"
                    inst["ldweights"] = True
                    lu = (pending.get("sync_info") or {}).get("on_update") or []
                    if lu:
                        inst["sync_info"]["on_update"].extend(lu)
                    pending = None
                out.append(inst)
            assert pending is None, "trailing Ldweights"
            blk["instructions"] = out
    return _json.dumps(bir).encode()


_orig_run_command = _bu.run_command


def _patched_run_command(argv, **kwargs):
    if isinstance(argv, list):
        argv = [
            "--enable-ldw-opt=true" if a == "--enable-ldw-opt=false" else a
            for a in argv
        ]
    return _orig_run_command(argv, **kwargs)


_bu.run_command = _patched_run_command

_orig_compile_impl = _bu._compile_bir_impl


def _patched_compile_impl(bir_json, *args, **kwargs):
    return _orig_compile_impl(_merge_ldweights(bir_json), *args, **kwargs)


_bu._compile_bir_impl = _patched_compile_impl

# ---------------------------------------------------------------------------

B, H, W, C = 8, 48, 192, 128
DK = C // 8               # 16
N = H * W                 # 9216 pixels
NT = N // 128             # 72 q tiles
NHALF = 2 * NT            # 144 half-tiles of 9 kpix slots each
KT = 18                   # pooled-pixel tiles of 128
NP = KT * 128             # 2304 pooled pixels
HP, WP = H // 2, W // 2   # 24, 96
VSLOT = 160               # vpool slot stride: 128 ch + 1 ones + pad
                          # (32-aligned: the XBAR transpose DMA corrupts
                          # non-32-aligned destination column offsets)

# Schraudolph exp: scores arrive pre-scaled by C1 = 2^7/ln2 (folded into
# Wq host-side).  DVE computes int16(s' + C2); those bits read as bf16 give
# exp(s) to within ~3%.  ACT computes exact exp via scale=1/C1.
C1 = 128.0 / float(np.log(2.0))
C2 = 127.0 * 128.0 - 5.5
ACT_SLOTS = 5                      # slots 0-4 exact exp on ACT
ACT_COLS = ACT_SLOTS * 128         # 640; DVE takes cols 640:1152

F32 = mybir.dt.float32
F16 = mybir.dt.float16
I16 = mybir.dt.int16
BF16 = mybir.dt.bfloat16

# slot/band layout of the 9 score tiles inside one [128, 1152] fp32 PSUM
# half.  Concurrent row-tiled matmuls must land in different 2KB PSUM
# banks (slot s lives in bank s//4).  The first three tiles run as a
# 3-way concurrent group on row bands 0/64/32 (k and q are replicated on
# partitions 0-15, 32-47, 64-79) hitting banks 0/1/2; the remaining six
# run as 2-way pairs on bands 0/64 hitting banks 0/1.
_SLOT = {0: 0, 1: 4, 2: 8, 3: 1, 4: 5, 5: 2, 6: 6, 7: 3, 8: 7}
_BAND = {0: 0, 1: 64, 2: 32, 3: 0, 4: 64, 5: 0, 6: 64, 7: 0, 8: 64}


def _build_nc(kw_fold=True):
    # kw_fold (valid when bq == 0, which the harness always supplies):
    # fold the q-projection into the score matmuls via KWT = C1*Wq@k^T
    # computed on-device.  Scores become full-K (K=128) matmuls: unlike
    # the K=16 row-group-masked form, these register as PE activity in
    # the HAM clock gate, so the steady loop can un-throttle to 2.4 GHz.
    # Also removes the q projection + its 18 ACT evacuations.
    nc = bass.Bass(target_bir_lowering=False)

    x32 = nc.dram_tensor("x32", [N, C], F32, kind="ExternalInput")
    # x transposed [C, N] on the HOST: a plain DMA loads it (Tile
    # serializes every xbar-transpose DMA against all other in-flight
    # DMAs -- the HW-deadlock guard -- which stretched the 6 transposed
    # xT chunk loads across ~40us of the preamble).
    xt_d = nc.dram_tensor("xt", [C, N], BF16, kind="ExternalInput")
    wv_d = nc.dram_tensor("wv", [C, C], BF16, kind="ExternalInput")
    if kw_fold:
        wqt_d = nc.dram_tensor("wqt", [DK, C], BF16, kind="ExternalInput")
    else:
        wq2_d = nc.dram_tensor("wq2", [C, 80], BF16, kind="ExternalInput")
        bq2_d = nc.dram_tensor("bq2", [80, 1], F32, kind="ExternalInput")
    wk2_d = nc.dram_tensor("wk2", [C, 80], BF16, kind="ExternalInput")
    if not kw_fold:
        beta_d = nc.dram_tensor("beta128", [C, 1], F32, kind="ExternalInput")
    out_d = nc.dram_tensor("out", [N, C], F32, kind="ExternalOutput")

    act_cols = ACT_COLS

    dma = nc.default_dma_engine

    with TileContext(nc) as tc:
        with (
            tc.tile_pool(name="const", bufs=1) as cpool,
            tc.tile_pool(name="big", bufs=1) as big,
        ):
            wv_sb = cpool.tile([C, C], BF16)
            wk2_sb = cpool.tile([C, 80], BF16)
            loads = [(wv_sb, wv_d), (wk2_sb, wk2_d)]
            if kw_fold:
                wqt_sb = cpool.tile([DK, C], BF16)
                loads.append((wqt_sb, wqt_d))
            else:
                beta_sb = cpool.tile([C, 1], F32)
                loads.append((beta_sb, beta_d))
                wq2_sb = cpool.tile([C, 80], BF16)
                bq2_sb = cpool.tile([80, 1], F32)
                loads += [(wq2_sb, wq2_d), (bq2_sb, bq2_d)]
            for sb, d in loads:
                dma.dma_start(sb[:, :], d[:, :])

            x_nat = big.tile([128, NT, 128], F32)
            xT = big.tile([128, N], BF16)
            if kw_fold:
                kwt_sb = big.tile([128, NP], BF16)
            else:
                qT = big.tile([128, N], F16)
            kpool = big.tile([128, NP], F16)
            vpoolT = big.tile([128, NP], BF16)
            vaug = big.tile([128, KT, VSLOT], BF16)

            # xT first (it gates every projection matmul), chunked so
            # the first k matmul can start early.
            for c6 in range(6):
                dma.dma_start(
                    xT[:, c6 * 1536:(c6 + 1) * 1536],
                    xt_d[:, c6 * 1536:(c6 + 1) * 1536],
                )
            # x (natural fp32, with beta*bv folded in host-side) is only
            # read by the residual adds at the end of each q tile.
            for c6 in range(6):
                dma.dma_start(
                    x_nat[:, c6 * 12:(c6 + 1) * 12, :],
                    x32[c6 * 12 * 128:(c6 + 1) * 12 * 128, :].rearrange(
                        "(t p) c -> p t c", p=128
                    ),
                )

            # ones column for the softmax-denominator trick; transposed V
            # tiles later overwrite cols 0-127 of each slot.
            nc.vector.memset(vaug[:, :, :], 1.0)

            # ---- projections + pooling --------------------------------
            # PE: k, v, q matmuls.  DVE: maxpool reductions straight from
            # PSUM.  ACT: q evacuation with bias folded.  DMA: transposes
            # pooled V into vaug [kpix, C] slots.
            # bk needs no add at all: softmax(q.(k+bk)) == softmax(q.k)
            # (a per-row constant shift).  bv is folded into the residual
            # host-side (out = (x + beta*bv) + beta*(o/d)).
            with (
                tc.tile_pool(name="ppsum", bufs=3, space="PSUM") as ppool,
                tc.tile_pool(name="qpsum", bufs=3, space="PSUM") as qpool,
            ):
                for ch in range(HP):
                    pk = ppool.tile([128, 2 * W], F32, tag="pp")
                    nc.tensor.matmul(
                        pk[0:80, :], wk2_sb[:, :],
                        xT[:, ch * 2 * W:(ch + 1) * 2 * W],
                        start=True, stop=True,
                    )
                    nc.vector.tensor_reduce(
                        out=kpool[0:80, ch * WP:(ch + 1) * WP],
                        in_=pk[0:80, :].rearrange(
                            "p (hh w2 wp) -> p w2 hh wp", hh=2, wp=2
                        ),
                        axis=mybir.AxisListType.XY,
                        op=mybir.AluOpType.max,
                    )

                for ch in range(HP):
                    pv = ppool.tile([128, 2 * W], F32, tag="pp")
                    nc.tensor.matmul(
                        pv[:, :], wv_sb[:, :],
                        xT[:, ch * 2 * W:(ch + 1) * 2 * W],
                        start=True, stop=True,
                    )
                    nc.vector.tensor_reduce(
                        out=vpoolT[:, ch * WP:(ch + 1) * WP],
                        in_=pv[:, :].rearrange(
                            "p (hh w2 wp) -> p w2 hh wp", hh=2, wp=2
                        ),
                        axis=mybir.AxisListType.XY,
                        op=mybir.AluOpType.max,
                    )

                if kw_fold:
                    # KWT[c, kpix] = C1 * Wq @ k^T, evacuated bf16; the
                    # scores are then KWT_t^T @ xT with K=128.
                    for off in range(0, NP, 512):
                        w = min(512, NP - off)
                        pq = qpool.tile([128, 512], F32, tag="pq")
                        nc.tensor.matmul(
                            pq[:, 0:w], wqt_sb[:, :],
                            kpool[0:DK, off:off + w],
                            start=True, stop=True,
                        )
                        nc.scalar.activation(
                            out=kwt_sb[:, off:off + w], in_=pq[:, 0:w],
                            func=mybir.ActivationFunctionType.Identity,
                        )
                else:
                    for ch in range(KT):
                        pq = qpool.tile([128, 512], F32, tag="pq")
                        nc.tensor.matmul(
                            pq[0:80, :], wq2_sb[:, :],
                            xT[:, ch * 512:(ch + 1) * 512],
                            start=True, stop=True,
                        )
                        nc.scalar.activation(
                            out=qT[0:80, ch * 512:(ch + 1) * 512],
                            in_=pq[0:80, :],
                            func=mybir.ActivationFunctionType.Identity,
                            bias=bq2_sb[:, 0:1],
                        )

                # transpose pooled V into [kpix, C] tiles on the DMA engine
                for t in range(KT):
                    dma.dma_start_transpose(
                        vaug[:, t, 0:128],
                        vpoolT[:, t * 128:(t + 1) * 128],
                    )

            # ---- attention main loop ----------------------------------
            # Software-pipelined with 2-half lookahead: the PE stream is
            # s(0) s(1) s(2) pv(0) s(3) pv(1) ... with no dependency
            # stalls, so the HAM clock gate keeps the PE at 2.4 GHz.
            # exp is ACT-only for the first halves (DVE is still draining
            # the preamble maxpool reductions), then splits ACT/DVE.
            # finish: DVE computes (o * 1/d) * beta, GpSimd adds the
            # residual (x already carries beta*bv), keeping DVE light.
            ACT_ONLY = 8
            with (
                tc.tile_pool(name="spsum", bufs=3, space="PSUM") as spool,
                tc.tile_pool(name="spsum1", bufs=1, space="PSUM") as spool1,
                tc.tile_pool(name="opsum", bufs=1, space="PSUM") as opool,
                tc.tile_pool(name="expp", bufs=4) as epool,
                tc.tile_pool(name="outp", bufs=4) as outpool,
                tc.tile_pool(name="ob1p", bufs=4) as ob1pool,
                tc.tile_pool(name="sclp", bufs=4) as sclpool,
            ):
                sing = spool1.tile([128, 3 * 128], F32)

                def scores(h):
                    q, half = divmod(h, 2)
                    s_ps = spool.tile([128, 8 * 128], F32, tag="s")
                    for tt in range(9):
                        t = half * 9 + tt
                        slot, band = _SLOT[tt], _BAND[tt]
                        if slot == 8:
                            dst = sing[:, (h % 3) * 128:(h % 3 + 1) * 128]
                        else:
                            dst = s_ps[:, slot * 128:(slot + 1) * 128]
                        if kw_fold:
                            nc.tensor.matmul(
                                dst,
                                kwt_sb[:, t * 128:(t + 1) * 128],
                                xT[:, q * 128:(q + 1) * 128],
                                start=True, stop=True,
                            )
                        else:
                            nc.tensor.matmul(
                                dst,
                                kpool[band:band + 16, t * 128:(t + 1) * 128],
                                qT[band:band + 16, q * 128:(q + 1) * 128],
                                start=True, stop=True,
                                tile_position=(band, 0),
                            )
                    return s_ps

                def exp_half(h, s_ps):
                    # high_priority: when an exp and a finish op are both
                    # ready on ACT/DVE, schedule the exp first -- a finish
                    # op ahead of exp in the FIFO stalls the PE's scores.
                    ctx = tc.high_priority()
                    ctx.__enter__()
                    e = epool.tile([128, 9 * 128], BF16, tag="exp")
                    ss = sing[:, (h % 3) * 128:(h % 3 + 1) * 128]
                    if h < ACT_ONLY:
                        nc.scalar.activation(
                            out=e[:, 0:1024], in_=s_ps[:, 0:1024],
                            func=mybir.ActivationFunctionType.Exp,
                            scale=1.0 / C1,
                        )
                        nc.scalar.activation(
                            out=e[:, 1024:1152], in_=ss,
                            func=mybir.ActivationFunctionType.Exp,
                            scale=1.0 / C1,
                        )
                    else:
                        nc.scalar.activation(
                            out=e[:, 0:act_cols], in_=s_ps[:, 0:act_cols],
                            func=mybir.ActivationFunctionType.Exp,
                            scale=1.0 / C1,
                        )
                        nc.vector.tensor_scalar(
                            out=e[:, act_cols:1024].bitcast(I16),
                            in0=s_ps[:, act_cols:1024],
                            scalar1=C2, scalar2=None,
                            op0=mybir.AluOpType.add,
                        )
                        nc.vector.tensor_scalar(
                            out=e[:, 1024:1152].bitcast(I16),
                            in0=ss,
                            scalar1=C2, scalar2=None,
                            op0=mybir.AluOpType.add,
                        )
                    ctx.__exit__(None, None, None)
                    return e

                def pv(h, e, o_ps):
                    half = h % 2
                    for tt in range(9):
                        t = half * 9 + tt
                        slot = _SLOT[tt]
                        nc.tensor.matmul(
                            o_ps[:, 0:129],
                            e[:, slot * 128:(slot + 1) * 128],
                            vaug[:, t, 0:129],
                            start=(t == 0), stop=(t == KT - 1),
                        )

                def finish_kw(q, o_ps):
                    # beta is folded into Wv host-side, so the whole
                    # finish is: one DVE reciprocal of the denominator
                    # (PSUM; safe -- finish is emitted two halves after
                    # the pv, so its PV wait never head-of-line-blocks
                    # the DVE FIFO) + one fused ACT evacuate-normalize
                    # with a per-partition scale.
                    recip = sclpool.tile([128, 1], F32, tag="recip")
                    nc.vector.reciprocal(recip[:, :], o_ps[:, 128:129])
                    ob1 = ob1pool.tile([128, 128], F32, tag="ob1")
                    nc.scalar.activation(
                        out=ob1[:, :], in_=o_ps[:, 0:128],
                        func=mybir.ActivationFunctionType.Identity,
                        scale=recip[:, 0:1],
                    )
                    ob = outpool.tile([128, 128], F32, tag="ob")
                    nc.gpsimd.tensor_tensor(
                        out=ob[:, :], in0=ob1[:, :], in1=x_nat[:, q, :],
                        op=mybir.AluOpType.add,
                    )
                    dma.dma_start(out_d[q * 128:(q + 1) * 128, :], ob[:, :])

                def finish(q, o_ps):
                    if kw_fold:
                        return finish_kw(q, o_ps)
                    # ACT (which trails the PV by design) evacuates o to
                    # SBUF so no DVE op ever waits on PV completion --
                    # a PV-dependent op at the DVE FIFO head would starve
                    # the next half's exp and stall the PE.
                    osb = ob1pool.tile([128, 129], F32, tag="osb")
                    nc.scalar.activation(
                        out=osb[:, :], in_=o_ps[:, 0:129],
                        func=mybir.ActivationFunctionType.Copy,
                    )
                    # normalize on DVE (reads SBUF only, so it never waits
                    # on PV at the DVE FIFO head); residual add on GpSimd.
                    recip = sclpool.tile([128, 1], F32, tag="recip")
                    nc.vector.reciprocal(recip[:, :], osb[:, 128:129])
                    ob1 = ob1pool.tile([128, 128], F32, tag="ob1")
                    nc.vector.tensor_scalar(
                        out=ob1[:, :], in0=osb[:, 0:128],
                        scalar1=recip[:, 0:1], scalar2=beta_sb[:, 0:1],
                        op0=mybir.AluOpType.mult, op1=mybir.AluOpType.mult,
                    )
                    ob = outpool.tile([128, 128], F32, tag="ob")
                    nc.gpsimd.tensor_tensor(
                        out=ob[:, :], in0=ob1[:, :], in1=x_nat[:, q, :],
                        op=mybir.AluOpType.add,
                    )
                    dma.dma_start(out_d[q * 128:(q + 1) * 128, :], ob[:, :])

                # finish(q) is emitted two halves AFTER pv(2q+1) so its
                # pv-completion wait never head-of-line-blocks the next
                # half's DVE exp (strict 8-deep FIFO).
                es = []
                o_tiles = {}
                for h in range(NHALF + 4):
                    if h < NHALF:
                        s_ps = scores(h)
                        es.append(exp_half(h, s_ps))
                    hp = h - 2
                    if 0 <= hp < NHALF:
                        if hp % 2 == 0:
                            o_tiles[hp // 2] = opool.tile(
                                [128, 129], F32, tag="o", name="o_ps"
                            )
                        pv(hp, es.pop(0), o_tiles[hp // 2])
                    if h >= 5 and (h - 5) % 2 == 0:
                        q = (h - 5) // 2
                        finish(q, o_tiles.pop(q))

    return nc


_CACHE = {}


def _get_nc(kw_fold):
    key = f"nc{int(kw_fold)}"
    if key not in _CACHE:
        _CACHE[key] = _build_nc(kw_fold)
    return _CACHE[key]


def kernel(query, Wq, bq, Wk, bk, Wv, bv, beta, _trace=False):
    query = np.ascontiguousarray(np.asarray(query, dtype=np.float32))
    Wq = np.asarray(Wq, dtype=np.float32)
    Wk = np.asarray(Wk, dtype=np.float32)
    Wv = np.asarray(Wv, dtype=np.float32)
    bq = np.asarray(bq, dtype=np.float32)
    bk = np.asarray(bk, dtype=np.float32)
    bv = np.asarray(bv, dtype=np.float32)
    beta = np.asarray(beta, dtype=np.float32)

    # replicate q/k weights into PE columns 0-15, 32-47 and 64-79 so
    # their projections land on the row-tiled sub-arrays' SBUF
    # partitions (bands 0/32/64 for the 3-way score group).  Wq/bq are
    # additionally scaled by C1 so the scores arrive pre-scaled for the
    # Schraudolph exp (see module docstring).
    wq2 = np.zeros((C, 80), dtype=np.float32)
    wk2 = np.zeros((C, 80), dtype=np.float32)
    bq2 = np.zeros((80, 1), dtype=np.float32)
    for off in (0, 32, 64):
        wq2[:, off:off + DK] = Wq * C1
        wk2[:, off:off + DK] = Wk
        bq2[off:off + DK, 0] = bq * C1
    # bk never needs to be applied: softmax(q.(k+bk)) == softmax(q.k)
    # (it shifts every logit in a row by the same q.bk constant).

    beta_s = float(beta.ravel()[0])
    # bq == 0 (always, for this harness) enables the kw_fold graph: the
    # q projection folds into the score matmuls via KWT = C1*Wq@k^T.
    kw_fold = not np.any(bq)
    shared = {
        "wk2": wk2.astype(ml_dtypes.bfloat16),
    }
    if kw_fold:
        # beta folds into Wv (o' = s @ (v*beta) = beta*(s@v)); the
        # denominator (ones column) is unaffected.
        shared["wv"] = (Wv * beta_s).astype(ml_dtypes.bfloat16)
        shared["wqt"] = np.ascontiguousarray(
            (Wq * C1).T.astype(ml_dtypes.bfloat16)
        )
    else:
        shared["wv"] = Wv.astype(ml_dtypes.bfloat16)
        shared["beta128"] = np.full((C, 1), beta_s, dtype=np.float32)
        shared["wq2"] = wq2.astype(ml_dtypes.bfloat16)
        shared["bq2"] = bq2
    # fold beta*bv into the residual: out = (x + beta*bv) + beta*(o/d)
    bv_fold = (beta_s * bv).astype(np.float32)[None, :]
    in_maps = []
    for b in range(B):
        img = query[b].reshape(N, C)
        m = dict(shared)
        m["x32"] = img + bv_fold
        m["xt"] = np.ascontiguousarray(img.T.astype(ml_dtypes.bfloat16))
        in_maps.append(m)

    nc = _get_nc(kw_fold)
    res = run_bass_kernel_spmd(
        nc, in_maps, core_ids=list(range(B)), trace=_trace
    )
    out = np.stack([res.results[b]["out"] for b in range(B)])
    out = out.reshape(B, H, W, C).astype(np.float32)
    if _trace:
        kernel.last_exec_time_ns = res.exec_time_ns
        kernel.last_profile_json = res.profile_json
    return out



# revision 45
# speedup vs baseline: 1.1152x; 1.1152x over previous
"""Trainium2 Bass kernel for ConditionalAttentionConv2D.

Reference computation (per image, B=8 images total):
    k = maxpool2x2(x @ Wk + bk)          [2304, 16]
    q = x @ Wq + bq                      [9216, 16]
    s = softmax(q @ k^T, axis=-1)        [9216, 2304]
    v = maxpool2x2(x @ Wv + bv)          [2304, 128]
    out = x + beta * (s @ v)             [9216, 128]

Sharding: data-parallel over batch, one image per NeuronCore (8 cores).

Key performance structure:
  - The attention loop is software-pipelined with a 2-half lookahead
    (scores run two half-tiles ahead of the PV matmuls) to minimize PE
    dependency stalls (the HAM clock gate throttles the PE to 1.2 GHz
    whenever its activity window sees idle gaps).
  - exp is split across two engines so it keeps pace with the PE: ACT
    computes exact exp on slots 0-4 (with scale=1/C1), DVE computes a
    Schraudolph-style exp on slots 5-8: scores arrive pre-scaled by
    C1 = 2^7*log2(e) (folded into Wq on the host), DVE adds the exponent
    bias C2 and converts fp32->int16, whose bits ARE the bf16 exp value.
    The softmax denominator (ones column appended to V) sums the actual
    e values used, so rows still normalize exactly.  The first halves are
    ACT-only while DVE drains the preamble maxpool reductions.
  - The first three score tiles of each half run as a 3-way concurrent
    row-tiled group (bands 0/64/32, PSUM banks 0/1/2); the rest as 2-way
    pairs.  k/q are replicated on partitions 0-15, 32-47, 64-79.
  - bk is never applied (softmax is invariant to the per-row constant
    q.bk); beta*bv is folded into the residual input host-side.
  - finish: ACT evacuates o from PSUM, DVE normalizes from SBUF, GpSimd
    adds the residual -- no DVE op ever waits on PV completion, which
    would head-of-line-block the next half's exp in the DVE FIFO.
  - Walrus's LDW optimization is enabled by re-merging tile_legalize's
    standalone Ldweights into their Matmults at the BIR-JSON level (see
    _merge_ldweights).
  - Preamble: xT is transposed on the HOST and loaded by plain DMA
    chunks (Tile serializes every xbar-transpose DMA against all other
    in-flight DMAs -- the HW-deadlock guard -- so transposed loads
    stretched across ~40us of the preamble); pooled-V tiles are
    transposed into vaug by the DMA engine (slot stride 160: the XBAR
    transpose DMA corrupts non-32-aligned destination offsets).
"""

import os
import numpy as np

import ml_dtypes

from concourse import bass, mybir, masks
from concourse.tile import TileContext
from concourse.bass_utils import run_bass_kernel_spmd

# ---------------------------------------------------------------------------
# Walrus in this toolchain rejects >1 sync-wait on a CTRL instruction, but
# TileContext's final drain carries one wait per active proc.  Split them
# across standalone sync-engine NOPs before a bare drain.
# ---------------------------------------------------------------------------


def _patched_drain_and_barrier(self, tick_clock, wait_clock):
    from concourse.vector_clock import ScopedClock

    nop_inst = self.nc.sync.nop(nofuse=True)
    wait_clock.add_sem_waits(
        nop_inst.ins, ScopedClock({None: tick_clock.global_clock})
    )
    si = nop_inst.ins.sync_info
    waits = list(si.on_wait) if si is not None else []
    if len(waits) > 1:
        del si.on_wait[1:]
        for w in waits[1:]:
            n2 = self.nc.sync.nop(nofuse=True)
            n2.ins.sync_info = mybir.SyncInfo(on_wait=[w], on_update=[])
    self.nc.sync.drain()
    self.nc.all_engine_barrier()
    popped = self.nc._tile_sem_poison_stack.pop()
    assert popped is self._sem_poison
    self.nc.clear_and_free_semaphores(list(self.sems.allocated().values()))
    self.nc.all_engine_barrier()


TileContext._drain_and_barrier = _patched_drain_and_barrier


def _tile_structural_classes():
    import concourse.tile as _t

    names = (
        "BassTileCriticalSection", "BassTileConditionalBlock",
        "TileBranchInst", "BassTileRelease",
        "BassTileBranchHintPlaceholder", "BassTileLoopBlock",
    )
    return tuple(getattr(_t, n) for n in names if hasattr(_t, n))


_STRUCTURAL = None
_orig_commit_and_lower = TileContext._commit_and_lower


def _patched_commit_and_lower(self, inst, original_block, old_bb_map,
                              bb_to_exit_bb):
    global _STRUCTURAL
    if _STRUCTURAL is None:
        _STRUCTURAL = _tile_structural_classes()
    si = getattr(inst, "sync_info", None)
    # Ldweights cannot carry waits under walrus's LDW optimization, and no
    # instruction can carry more than one wait on this toolchain: hoist the
    # excess onto same-engine NOPs committed immediately before.
    keep = 0 if isinstance(inst, mybir.InstLdweights) else 1
    if (
        si is not None
        and si.on_wait
        and len(si.on_wait) > keep
        and not isinstance(inst, _STRUCTURAL)
    ):
        waits = list(si.on_wait)
        si.on_wait[:] = waits[len(waits) - keep:]
        for i, w in enumerate(waits[:len(waits) - keep]):
            nop = mybir.InstNoOp(
                name=f"{inst.name}-sw{i}",
                engine=inst.engine,
                sync_info=mybir.SyncInfo(on_wait=[w], on_update=[]),
                bass_nofuse=True,
            )
            self._commit_instruction(nop)
    return _orig_commit_and_lower(
        self, inst, original_block, old_bb_map, bb_to_exit_bb
    )


TileContext._commit_and_lower = _patched_commit_and_lower

# ---------------------------------------------------------------------------
# Enable walrus's LDW optimization (incl. Fast Weight Load: 2x bf16
# weight-load bandwidth).  tile_legalize always splits InstMatmult into a
# standalone InstLdweights + a non-self-loading InstMatmult, which walrus
# rejects under --enable-ldw-opt.  So just before walrus runs, merge each
# Ldweights back into its Matmult (ldweights=true) at the BIR-JSON level
# and flip the flag.  The TileContext commit patch above already keeps
# sem-waits OFF the Ldweights instructions (hoisted to NOPs), so the merge
# only has to migrate on_update entries.
# ---------------------------------------------------------------------------

import json as _json

from concourse import bass_utils as _bu
from concourse import hw_specs as _hw

# ---------------------------------------------------------------------------
# Make the Tile scheduler's cost model see the PE at its HAM-throttled
# 1.2 GHz rate.  The default 2.4 GHz model makes the simulated PV matmuls
# finish early, so the scheduler places PV-dependent ops (o-evacuation,
# normalize) ahead of the next half's exp in the ACT/DVE FIFOs; on real
# (cold) hardware those ops then wait at the FIFO head and stall the PE.
# This only changes instruction scheduling, not hardware behavior.
# ---------------------------------------------------------------------------
_hw.TRN2Spec.PE_CYCLE = 1e9 / 1.2e9


def _merge_ldweights(bir_json: bytes) -> bytes:
    bir = _json.loads(bir_json)
    for fn in bir["functions"]:
        for blk in fn["blocks"]:
            out = []
            pending = None
            for inst in blk["instructions"]:
                op = inst.get("opcode")
                if op == "Ldweights":
                    assert pending is None, "two Ldweights without Matmult"
                    si = inst.get("sync_info") or {}
                    assert not si.get("on_wait"), (
                        f"LDW {inst['name']} carries waits"
                    )
                    pending = inst
                    continue
                if op == "Matmult" and pending is not None:
                    w = pending["ins"][0]
                    mw = inst["ins"][1]
                    assert (
                        w["memref"] == mw["memref"]
                        and w["offset"] == mw["offset"]
                    ), f"LDW/MM pairing mismatch {pending['name']} {inst['name']}"
                    inst["ldweights"] = True
                    lu = (pending.get("sync_info") or {}).get("on_update") or []
                    if lu:
                        inst["sync_info"]["on_update"].extend(lu)
                    pending = None
                out.append(inst)
            assert pending is None, "trailing Ldweights"
            blk["instructions"] = out
    return _json.dumps(bir).encode()


_orig_run_command = _bu.run_command


def _patched_run_command(argv, **kwargs):
    if isinstance(argv, list):
        argv = [
            "--enable-ldw-opt=true" if a == "--enable-ldw-opt=false" else a
            for a in argv
        ]
    return _orig_run_command(argv, **kwargs)


_bu.run_command = _patched_run_command

_orig_compile_impl = _bu._compile_bir_impl


def _patched_compile_impl(bir_json, *args, **kwargs):
    return _orig_compile_impl(_merge_ldweights(bir_json), *args, **kwargs)


_bu._compile_bir_impl = _patched_compile_impl

# ---------------------------------------------------------------------------

B, H, W, C = 8, 48, 192, 128
DK = C // 8               # 16
N = H * W                 # 9216 pixels
NT = N // 128             # 72 q tiles
NHALF = 2 * NT            # 144 half-tiles of 9 kpix slots each
KT = 18                   # pooled-pixel tiles of 128
NP = KT * 128             # 2304 pooled pixels
HP, WP = H // 2, W // 2   # 24, 96
VSLOT = 160               # vpool slot stride: 128 ch + 1 ones + pad
                          # (32-aligned: the XBAR transpose DMA corrupts
                          # non-32-aligned destination column offsets)

# Schraudolph exp: scores arrive pre-scaled by C1 = 2^7/ln2 (folded into
# Wq host-side).  DVE computes int16(s' + C2); those bits read as bf16 give
# exp(s) to within ~3%.  ACT computes exact exp via scale=1/C1.
C1 = 128.0 / float(np.log(2.0))
C2 = 127.0 * 128.0 - 5.5
ACT_SLOTS = 5                      # slots 0-4 exact exp on ACT
ACT_COLS = ACT_SLOTS * 128         # 640; DVE takes cols 640:1152

F32 = mybir.dt.float32
F16 = mybir.dt.float16
I16 = mybir.dt.int16
BF16 = mybir.dt.bfloat16

# slot/band layout of the 9 score tiles inside one [128, 1152] fp32 PSUM
# half.  Concurrent row-tiled matmuls must land in different 2KB PSUM
# banks (slot s lives in bank s//4).  The first three tiles run as a
# 3-way concurrent group on row bands 0/64/32 (k and q are replicated on
# partitions 0-15, 32-47, 64-79) hitting banks 0/1/2; the remaining six
# run as 2-way pairs on bands 0/64 hitting banks 0/1.
_SLOT = {0: 0, 1: 4, 2: 8, 3: 1, 4: 5, 5: 2, 6: 6, 7: 3, 8: 7}
_BAND = {0: 0, 1: 64, 2: 32, 3: 0, 4: 64, 5: 0, 6: 64, 7: 0, 8: 64}


def _build_nc(kw_fold=True):
    # kw_fold (valid when bq == 0, which the harness always supplies):
    # fold the q-projection into the score matmuls via KWT = C1*Wq@k^T
    # computed on-device.  Scores become full-K (K=128) matmuls: unlike
    # the K=16 row-group-masked form, these register as PE activity in
    # the HAM clock gate, so the steady loop can un-throttle to 2.4 GHz.
    # Also removes the q projection + its 18 ACT evacuations.
    nc = bass.Bass(target_bir_lowering=False)

    x32 = nc.dram_tensor("x32", [N, C], F32, kind="ExternalInput")
    # x transposed [C, N] on the HOST: a plain DMA loads it (Tile
    # serializes every xbar-transpose DMA against all other in-flight
    # DMAs -- the HW-deadlock guard -- which stretched the 6 transposed
    # xT chunk loads across ~40us of the preamble).
    xt_d = nc.dram_tensor("xt", [C, N], BF16, kind="ExternalInput")
    wv_d = nc.dram_tensor("wv", [C, C], BF16, kind="ExternalInput")
    if kw_fold:
        wqt_d = nc.dram_tensor("wqt", [DK, C], BF16, kind="ExternalInput")
    else:
        wq2_d = nc.dram_tensor("wq2", [C, 80], BF16, kind="ExternalInput")
        bq2_d = nc.dram_tensor("bq2", [80, 1], F32, kind="ExternalInput")
    wk2_d = nc.dram_tensor("wk2", [C, 80], BF16, kind="ExternalInput")
    beta_d = nc.dram_tensor("beta128", [C, 1], F32, kind="ExternalInput")
    out_d = nc.dram_tensor("out", [N, C], F32, kind="ExternalOutput")

    # exp engine split: with full-K scores the PE runs warm and ACT
    # becomes the pacer, so give DVE a bigger share (512/640 vs 640/512).
    act_cols = 512 if kw_fold else ACT_COLS

    dma = nc.default_dma_engine

    with TileContext(nc) as tc:
        with (
            tc.tile_pool(name="const", bufs=1) as cpool,
            tc.tile_pool(name="big", bufs=1) as big,
        ):
            wv_sb = cpool.tile([C, C], BF16)
            wk2_sb = cpool.tile([C, 80], BF16)
            beta_sb = cpool.tile([C, 1], F32)
            loads = [(wv_sb, wv_d), (wk2_sb, wk2_d), (beta_sb, beta_d)]
            if kw_fold:
                wqt_sb = cpool.tile([DK, C], BF16)
                loads.append((wqt_sb, wqt_d))
            else:
                wq2_sb = cpool.tile([C, 80], BF16)
                bq2_sb = cpool.tile([80, 1], F32)
                loads += [(wq2_sb, wq2_d), (bq2_sb, bq2_d)]
            for sb, d in loads:
                dma.dma_start(sb[:, :], d[:, :])

            x_nat = big.tile([128, NT, 128], F32)
            xT = big.tile([128, N], BF16)
            if kw_fold:
                kwt_sb = big.tile([128, NP], BF16)
            else:
                qT = big.tile([128, N], F16)
            kpool = big.tile([128, NP], F16)
            vpoolT = big.tile([128, NP], BF16)
            vaug = big.tile([128, KT, VSLOT], BF16)

            # xT first (it gates every projection matmul), chunked so
            # the first k matmul can start early.
            for c6 in range(6):
                dma.dma_start(
                    xT[:, c6 * 1536:(c6 + 1) * 1536],
                    xt_d[:, c6 * 1536:(c6 + 1) * 1536],
                )
            # x (natural fp32, with beta*bv folded in host-side) is only
            # read by the residual adds at the end of each q tile.
            for c6 in range(6):
                dma.dma_start(
                    x_nat[:, c6 * 12:(c6 + 1) * 12, :],
                    x32[c6 * 12 * 128:(c6 + 1) * 12 * 128, :].rearrange(
                        "(t p) c -> p t c", p=128
                    ),
                )

            # ones column for the softmax-denominator trick; transposed V
            # tiles later overwrite cols 0-127 of each slot.
            nc.vector.memset(vaug[:, :, :], 1.0)

            # ---- projections + pooling --------------------------------
            # PE: k, v, q matmuls.  DVE: maxpool reductions straight from
            # PSUM.  ACT: q evacuation with bias folded.  DMA: transposes
            # pooled V into vaug [kpix, C] slots.
            # bk needs no add at all: softmax(q.(k+bk)) == softmax(q.k)
            # (a per-row constant shift).  bv is folded into the residual
            # host-side (out = (x + beta*bv) + beta*(o/d)).
            with (
                tc.tile_pool(name="ppsum", bufs=3, space="PSUM") as ppool,
                tc.tile_pool(name="qpsum", bufs=3, space="PSUM") as qpool,
            ):
                for ch in range(HP):
                    pk = ppool.tile([128, 2 * W], F32, tag="pp")
                    nc.tensor.matmul(
                        pk[0:80, :], wk2_sb[:, :],
                        xT[:, ch * 2 * W:(ch + 1) * 2 * W],
                        start=True, stop=True,
                    )
                    nc.vector.tensor_reduce(
                        out=kpool[0:80, ch * WP:(ch + 1) * WP],
                        in_=pk[0:80, :].rearrange(
                            "p (hh w2 wp) -> p w2 hh wp", hh=2, wp=2
                        ),
                        axis=mybir.AxisListType.XY,
                        op=mybir.AluOpType.max,
                    )

                for ch in range(HP):
                    pv = ppool.tile([128, 2 * W], F32, tag="pp")
                    nc.tensor.matmul(
                        pv[:, :], wv_sb[:, :],
                        xT[:, ch * 2 * W:(ch + 1) * 2 * W],
                        start=True, stop=True,
                    )
                    nc.vector.tensor_reduce(
                        out=vpoolT[:, ch * WP:(ch + 1) * WP],
                        in_=pv[:, :].rearrange(
                            "p (hh w2 wp) -> p w2 hh wp", hh=2, wp=2
                        ),
                        axis=mybir.AxisListType.XY,
                        op=mybir.AluOpType.max,
                    )

                if kw_fold:
                    # KWT[c, kpix] = C1 * Wq @ k^T, evacuated bf16; the
                    # scores are then KWT_t^T @ xT with K=128.
                    for off in range(0, NP, 512):
                        w = min(512, NP - off)
                        pq = qpool.tile([128, 512], F32, tag="pq")
                        nc.tensor.matmul(
                            pq[:, 0:w], wqt_sb[:, :],
                            kpool[0:DK, off:off + w],
                            start=True, stop=True,
                        )
                        nc.scalar.activation(
                            out=kwt_sb[:, off:off + w], in_=pq[:, 0:w],
                            func=mybir.ActivationFunctionType.Identity,
                        )
                else:
                    for ch in range(KT):
                        pq = qpool.tile([128, 512], F32, tag="pq")
                        nc.tensor.matmul(
                            pq[0:80, :], wq2_sb[:, :],
                            xT[:, ch * 512:(ch + 1) * 512],
                            start=True, stop=True,
                        )
                        nc.scalar.activation(
                            out=qT[0:80, ch * 512:(ch + 1) * 512],
                            in_=pq[0:80, :],
                            func=mybir.ActivationFunctionType.Identity,
                            bias=bq2_sb[:, 0:1],
                        )

                # transpose pooled V into [kpix, C] tiles on the DMA engine
                for t in range(KT):
                    dma.dma_start_transpose(
                        vaug[:, t, 0:128],
                        vpoolT[:, t * 128:(t + 1) * 128],
                    )

            # ---- attention main loop ----------------------------------
            # Software-pipelined with 2-half lookahead: the PE stream is
            # s(0) s(1) s(2) pv(0) s(3) pv(1) ... with no dependency
            # stalls, so the HAM clock gate keeps the PE at 2.4 GHz.
            # exp is ACT-only for the first halves (DVE is still draining
            # the preamble maxpool reductions), then splits ACT/DVE.
            # finish: DVE computes (o * 1/d) * beta, GpSimd adds the
            # residual (x already carries beta*bv), keeping DVE light.
            ACT_ONLY = 8
            with (
                tc.tile_pool(name="spsum", bufs=3, space="PSUM") as spool,
                tc.tile_pool(name="spsum1", bufs=1, space="PSUM") as spool1,
                tc.tile_pool(name="opsum", bufs=1, space="PSUM") as opool,
                tc.tile_pool(name="expp", bufs=4) as epool,
                tc.tile_pool(name="outp", bufs=4) as outpool,
                tc.tile_pool(name="ob1p", bufs=4) as ob1pool,
                tc.tile_pool(name="sclp", bufs=4) as sclpool,
            ):
                sing = spool1.tile([128, 3 * 128], F32)

                def scores(h):
                    q, half = divmod(h, 2)
                    s_ps = spool.tile([128, 8 * 128], F32, tag="s")
                    for tt in range(9):
                        t = half * 9 + tt
                        slot, band = _SLOT[tt], _BAND[tt]
                        if slot == 8:
                            dst = sing[:, (h % 3) * 128:(h % 3 + 1) * 128]
                        else:
                            dst = s_ps[:, slot * 128:(slot + 1) * 128]
                        if kw_fold:
                            nc.tensor.matmul(
                                dst,
                                kwt_sb[:, t * 128:(t + 1) * 128],
                                xT[:, q * 128:(q + 1) * 128],
                                start=True, stop=True,
                            )
                        else:
                            nc.tensor.matmul(
                                dst,
                                kpool[band:band + 16, t * 128:(t + 1) * 128],
                                qT[band:band + 16, q * 128:(q + 1) * 128],
                                start=True, stop=True,
                                tile_position=(band, 0),
                            )
                    return s_ps

                def exp_half(h, s_ps):
                    # high_priority: when an exp and a finish op are both
                    # ready on ACT/DVE, schedule the exp first -- a finish
                    # op ahead of exp in the FIFO stalls the PE's scores.
                    ctx = tc.high_priority()
                    ctx.__enter__()
                    e = epool.tile([128, 9 * 128], BF16, tag="exp")
                    ss = sing[:, (h % 3) * 128:(h % 3 + 1) * 128]
                    if h < ACT_ONLY:
                        nc.scalar.activation(
                            out=e[:, 0:1024], in_=s_ps[:, 0:1024],
                            func=mybir.ActivationFunctionType.Exp,
                            scale=1.0 / C1,
                        )
                        nc.scalar.activation(
                            out=e[:, 1024:1152], in_=ss,
                            func=mybir.ActivationFunctionType.Exp,
                            scale=1.0 / C1,
                        )
                    else:
                        nc.scalar.activation(
                            out=e[:, 0:act_cols], in_=s_ps[:, 0:act_cols],
                            func=mybir.ActivationFunctionType.Exp,
                            scale=1.0 / C1,
                        )
                        nc.vector.tensor_scalar(
                            out=e[:, act_cols:1024].bitcast(I16),
                            in0=s_ps[:, act_cols:1024],
                            scalar1=C2, scalar2=None,
                            op0=mybir.AluOpType.add,
                        )
                        nc.vector.tensor_scalar(
                            out=e[:, 1024:1152].bitcast(I16),
                            in0=ss,
                            scalar1=C2, scalar2=None,
                            op0=mybir.AluOpType.add,
                        )
                    ctx.__exit__(None, None, None)
                    return e

                def pv(h, e, o_ps):
                    half = h % 2
                    for tt in range(9):
                        t = half * 9 + tt
                        slot = _SLOT[tt]
                        nc.tensor.matmul(
                            o_ps[:, 0:129],
                            e[:, slot * 128:(slot + 1) * 128],
                            vaug[:, t, 0:129],
                            start=(t == 0), stop=(t == KT - 1),
                        )

                def finish(q, o_ps):
                    # ACT (which trails the PV by design) evacuates o to
                    # SBUF so no DVE op ever waits on PV completion --
                    # a PV-dependent op at the DVE FIFO head would starve
                    # the next half's exp and stall the PE.
                    osb = ob1pool.tile([128, 129], F32, tag="osb")
                    nc.scalar.activation(
                        out=osb[:, :], in_=o_ps[:, 0:129],
                        func=mybir.ActivationFunctionType.Copy,
                    )
                    # reciprocal on DVE (reads SBUF only, so it never
                    # waits on PV at the DVE FIFO head); in kw_fold mode
                    # the normalize multiply runs on ACT (per-partition
                    # scale; beta already folded into Wv host-side) to
                    # offload the pacing DVE.  Residual add on GpSimd.
                    recip = sclpool.tile([128, 1], F32, tag="recip")
                    nc.vector.reciprocal(recip[:, :], osb[:, 128:129])
                    ob1 = ob1pool.tile([128, 128], F32, tag="ob1")
                    if kw_fold:
                        nc.scalar.activation(
                            out=ob1[:, :], in_=osb[:, 0:128],
                            func=mybir.ActivationFunctionType.Identity,
                            scale=recip[:, 0:1],
                        )
                    else:
                        nc.vector.tensor_scalar(
                            out=ob1[:, :], in0=osb[:, 0:128],
                            scalar1=recip[:, 0:1], scalar2=beta_sb[:, 0:1],
                            op0=mybir.AluOpType.mult, op1=mybir.AluOpType.mult,
                        )
                    ob = outpool.tile([128, 128], F32, tag="ob")
                    nc.gpsimd.tensor_tensor(
                        out=ob[:, :], in0=ob1[:, :], in1=x_nat[:, q, :],
                        op=mybir.AluOpType.add,
                    )
                    dma.dma_start(out_d[q * 128:(q + 1) * 128, :], ob[:, :])

                # finish(q) is emitted two halves AFTER pv(2q+1) so its
                # pv-completion wait never head-of-line-blocks the next
                # half's DVE exp (strict 8-deep FIFO).
                es = []
                o_tiles = {}
                for h in range(NHALF + 4):
                    if h < NHALF:
                        s_ps = scores(h)
                        es.append(exp_half(h, s_ps))
                    hp = h - 2
                    if 0 <= hp < NHALF:
                        if hp % 2 == 0:
                            o_tiles[hp // 2] = opool.tile(
                                [128, 129], F32, tag="o", name="o_ps"
                            )
                        pv(hp, es.pop(0), o_tiles[hp // 2])
                    if h >= 5 and (h - 5) % 2 == 0:
                        q = (h - 5) // 2
                        finish(q, o_tiles.pop(q))

    return nc


_CACHE = {}


def _get_nc(kw_fold):
    key = f"nc{int(kw_fold)}"
    if key not in _CACHE:
        _CACHE[key] = _build_nc(kw_fold)
    return _CACHE[key]


def kernel(query, Wq, bq, Wk, bk, Wv, bv, beta, _trace=False):
    query = np.ascontiguousarray(np.asarray(query, dtype=np.float32))
    Wq = np.asarray(Wq, dtype=np.float32)
    Wk = np.asarray(Wk, dtype=np.float32)
    Wv = np.asarray(Wv, dtype=np.float32)
    bq = np.asarray(bq, dtype=np.float32)
    bk = np.asarray(bk, dtype=np.float32)
    bv = np.asarray(bv, dtype=np.float32)
    beta = np.asarray(beta, dtype=np.float32)

    # replicate q/k weights into PE columns 0-15, 32-47 and 64-79 so
    # their projections land on the row-tiled sub-arrays' SBUF
    # partitions (bands 0/32/64 for the 3-way score group).  Wq/bq are
    # additionally scaled by C1 so the scores arrive pre-scaled for the
    # Schraudolph exp (see module docstring).
    wq2 = np.zeros((C, 80), dtype=np.float32)
    wk2 = np.zeros((C, 80), dtype=np.float32)
    bq2 = np.zeros((80, 1), dtype=np.float32)
    for off in (0, 32, 64):
        wq2[:, off:off + DK] = Wq * C1
        wk2[:, off:off + DK] = Wk
        bq2[off:off + DK, 0] = bq * C1
    # bk never needs to be applied: softmax(q.(k+bk)) == softmax(q.k)
    # (it shifts every logit in a row by the same q.bk constant).

    beta_s = float(beta.ravel()[0])
    # bq == 0 (always, for this harness) enables the kw_fold graph: the
    # q projection folds into the score matmuls via KWT = C1*Wq@k^T.
    kw_fold = not np.any(bq)
    shared = {
        # in kw_fold mode beta folds into Wv: o' = s @ (v*beta); the
        # denominator (ones column) is unaffected.
        "wv": ((Wv * beta_s) if kw_fold else Wv).astype(ml_dtypes.bfloat16),
        "wk2": wk2.astype(ml_dtypes.bfloat16),
        "beta128": np.full((C, 1), beta_s, dtype=np.float32),
    }
    if kw_fold:
        shared["wqt"] = np.ascontiguousarray(
            (Wq * C1).T.astype(ml_dtypes.bfloat16)
        )
    else:
        shared["wq2"] = wq2.astype(ml_dtypes.bfloat16)
        shared["bq2"] = bq2
    # fold beta*bv into the residual: out = (x + beta*bv) + beta*(o/d)
    bv_fold = (beta_s * bv).astype(np.float32)[None, :]
    in_maps = []
    for b in range(B):
        img = query[b].reshape(N, C)
        m = dict(shared)
        m["x32"] = img + bv_fold
        m["xt"] = np.ascontiguousarray(img.T.astype(ml_dtypes.bfloat16))
        in_maps.append(m)

    nc = _get_nc(kw_fold)
    res = run_bass_kernel_spmd(
        nc, in_maps, core_ids=list(range(B)), trace=_trace
    )
    out = np.stack([res.results[b]["out"] for b in range(B)])
    out = out.reshape(B, H, W, C).astype(np.float32)
    if _trace:
        kernel.last_exec_time_ns = res.exec_time_ns
        kernel.last_profile_json = res.profile_json
    return out



# revision 46
# speedup vs baseline: 1.3020x; 1.1675x over previous
"""Trainium2 Bass kernel for ConditionalAttentionConv2D.

Reference computation (per image, B=8 images total):
    k = maxpool2x2(x @ Wk + bk)          [2304, 16]
    q = x @ Wq + bq                      [9216, 16]
    s = softmax(q @ k^T, axis=-1)        [9216, 2304]
    v = maxpool2x2(x @ Wv + bv)          [2304, 128]
    out = x + beta * (s @ v)             [9216, 128]

Sharding: data-parallel over batch, one image per NeuronCore (8 cores).

Key performance structure:
  - The attention loop is software-pipelined with a 2-half lookahead
    (scores run two half-tiles ahead of the PV matmuls) to minimize PE
    dependency stalls (the HAM clock gate throttles the PE to 1.2 GHz
    whenever its activity window sees idle gaps).
  - exp is split across two engines so it keeps pace with the PE: ACT
    computes exact exp on slots 0-4 (with scale=1/C1), DVE computes a
    Schraudolph-style exp on slots 5-8: scores arrive pre-scaled by
    C1 = 2^7*log2(e) (folded into Wq on the host), DVE adds the exponent
    bias C2 and converts fp32->int16, whose bits ARE the bf16 exp value.
    The softmax denominator (ones column appended to V) sums the actual
    e values used, so rows still normalize exactly.  The first halves are
    ACT-only while DVE drains the preamble maxpool reductions.
  - The first three score tiles of each half run as a 3-way concurrent
    row-tiled group (bands 0/64/32, PSUM banks 0/1/2); the rest as 2-way
    pairs.  k/q are replicated on partitions 0-15, 32-47, 64-79.
  - bk is never applied (softmax is invariant to the per-row constant
    q.bk); beta*bv is folded into the residual input host-side.
  - finish: ACT evacuates o from PSUM, DVE normalizes from SBUF, GpSimd
    adds the residual -- no DVE op ever waits on PV completion, which
    would head-of-line-block the next half's exp in the DVE FIFO.
  - Walrus's LDW optimization is enabled by re-merging tile_legalize's
    standalone Ldweights into their Matmults at the BIR-JSON level (see
    _merge_ldweights).
  - Preamble: xT is transposed on the HOST and loaded by plain DMA
    chunks (Tile serializes every xbar-transpose DMA against all other
    in-flight DMAs -- the HW-deadlock guard -- so transposed loads
    stretched across ~40us of the preamble); pooled-V tiles are
    transposed into vaug by the DMA engine (slot stride 160: the XBAR
    transpose DMA corrupts non-32-aligned destination offsets).
"""

import os
import numpy as np

import ml_dtypes

from concourse import bass, mybir, masks
from concourse.tile import TileContext
from concourse.bass_utils import run_bass_kernel_spmd

# ---------------------------------------------------------------------------
# Walrus in this toolchain rejects >1 sync-wait on a CTRL instruction, but
# TileContext's final drain carries one wait per active proc.  Split them
# across standalone sync-engine NOPs before a bare drain.
# ---------------------------------------------------------------------------


def _patched_drain_and_barrier(self, tick_clock, wait_clock):
    from concourse.vector_clock import ScopedClock

    nop_inst = self.nc.sync.nop(nofuse=True)
    wait_clock.add_sem_waits(
        nop_inst.ins, ScopedClock({None: tick_clock.global_clock})
    )
    si = nop_inst.ins.sync_info
    waits = list(si.on_wait) if si is not None else []
    if len(waits) > 1:
        del si.on_wait[1:]
        for w in waits[1:]:
            n2 = self.nc.sync.nop(nofuse=True)
            n2.ins.sync_info = mybir.SyncInfo(on_wait=[w], on_update=[])
    self.nc.sync.drain()
    self.nc.all_engine_barrier()
    popped = self.nc._tile_sem_poison_stack.pop()
    assert popped is self._sem_poison
    self.nc.clear_and_free_semaphores(list(self.sems.allocated().values()))
    self.nc.all_engine_barrier()


TileContext._drain_and_barrier = _patched_drain_and_barrier


def _tile_structural_classes():
    import concourse.tile as _t

    names = (
        "BassTileCriticalSection", "BassTileConditionalBlock",
        "TileBranchInst", "BassTileRelease",
        "BassTileBranchHintPlaceholder", "BassTileLoopBlock",
    )
    return tuple(getattr(_t, n) for n in names if hasattr(_t, n))


_STRUCTURAL = None
_orig_commit_and_lower = TileContext._commit_and_lower


def _patched_commit_and_lower(self, inst, original_block, old_bb_map,
                              bb_to_exit_bb):
    global _STRUCTURAL
    if _STRUCTURAL is None:
        _STRUCTURAL = _tile_structural_classes()
    si = getattr(inst, "sync_info", None)
    # Ldweights cannot carry waits under walrus's LDW optimization, and no
    # instruction can carry more than one wait on this toolchain: hoist the
    # excess onto same-engine NOPs committed immediately before.
    keep = 0 if isinstance(inst, mybir.InstLdweights) else 1
    if (
        si is not None
        and si.on_wait
        and len(si.on_wait) > keep
        and not isinstance(inst, _STRUCTURAL)
    ):
        waits = list(si.on_wait)
        si.on_wait[:] = waits[len(waits) - keep:]
        for i, w in enumerate(waits[:len(waits) - keep]):
            nop = mybir.InstNoOp(
                name=f"{inst.name}-sw{i}",
                engine=inst.engine,
                sync_info=mybir.SyncInfo(on_wait=[w], on_update=[]),
                bass_nofuse=True,
            )
            self._commit_instruction(nop)
    return _orig_commit_and_lower(
        self, inst, original_block, old_bb_map, bb_to_exit_bb
    )


TileContext._commit_and_lower = _patched_commit_and_lower

# ---------------------------------------------------------------------------
# Enable walrus's LDW optimization (incl. Fast Weight Load: 2x bf16
# weight-load bandwidth).  tile_legalize always splits InstMatmult into a
# standalone InstLdweights + a non-self-loading InstMatmult, which walrus
# rejects under --enable-ldw-opt.  So just before walrus runs, merge each
# Ldweights back into its Matmult (ldweights=true) at the BIR-JSON level
# and flip the flag.  The TileContext commit patch above already keeps
# sem-waits OFF the Ldweights instructions (hoisted to NOPs), so the merge
# only has to migrate on_update entries.
# ---------------------------------------------------------------------------

import json as _json

from concourse import bass_utils as _bu
from concourse import hw_specs as _hw

# ---------------------------------------------------------------------------
# Make the Tile scheduler's cost model see the PE at its HAM-throttled
# 1.2 GHz rate.  The default 2.4 GHz model makes the simulated PV matmuls
# finish early, so the scheduler places PV-dependent ops (o-evacuation,
# normalize) ahead of the next half's exp in the ACT/DVE FIFOs; on real
# (cold) hardware those ops then wait at the FIFO head and stall the PE.
# This only changes instruction scheduling, not hardware behavior.
# ---------------------------------------------------------------------------
_hw.TRN2Spec.PE_CYCLE = 1e9 / 1.2e9


def _merge_ldweights(bir_json: bytes) -> bytes:
    bir = _json.loads(bir_json)
    for fn in bir["functions"]:
        for blk in fn["blocks"]:
            out = []
            pending = None
            for inst in blk["instructions"]:
                op = inst.get("opcode")
                if op == "Ldweights":
                    assert pending is None, "two Ldweights without Matmult"
                    si = inst.get("sync_info") or {}
                    assert not si.get("on_wait"), (
                        f"LDW {inst['name']} carries waits"
                    )
                    pending = inst
                    continue
                if op == "Matmult" and pending is not None:
                    w = pending["ins"][0]
                    mw = inst["ins"][1]
                    assert (
                        w["memref"] == mw["memref"]
                        and w["offset"] == mw["offset"]
                    ), f"LDW/MM pairing mismatch {pending['name']} {inst['name']}"
                    inst["ldweights"] = True
                    lu = (pending.get("sync_info") or {}).get("on_update") or []
                    if lu:
                        inst["sync_info"]["on_update"].extend(lu)
                    pending = None
                out.append(inst)
            assert pending is None, "trailing Ldweights"
            blk["instructions"] = out
    return _json.dumps(bir).encode()


_orig_run_command = _bu.run_command


def _patched_run_command(argv, **kwargs):
    if isinstance(argv, list):
        argv = [
            "--enable-ldw-opt=true" if a == "--enable-ldw-opt=false" else a
            for a in argv
        ]
    return _orig_run_command(argv, **kwargs)


_bu.run_command = _patched_run_command

_orig_compile_impl = _bu._compile_bir_impl


def _patched_compile_impl(bir_json, *args, **kwargs):
    return _orig_compile_impl(_merge_ldweights(bir_json), *args, **kwargs)


_bu._compile_bir_impl = _patched_compile_impl

# ---------------------------------------------------------------------------

B, H, W, C = 8, 48, 192, 128
DK = C // 8               # 16
N = H * W                 # 9216 pixels
NT = N // 128             # 72 q tiles
NHALF = 2 * NT            # 144 half-tiles of 9 kpix slots each
KT = 18                   # pooled-pixel tiles of 128
NP = KT * 128             # 2304 pooled pixels
HP, WP = H // 2, W // 2   # 24, 96
VSLOT = 160               # vpool slot stride: 128 ch + 1 ones + pad
                          # (32-aligned: the XBAR transpose DMA corrupts
                          # non-32-aligned destination column offsets)

# Schraudolph exp: scores arrive pre-scaled by C1 = 2^7/ln2 (folded into
# Wq host-side).  DVE computes int16(s' + C2); those bits read as bf16 give
# exp(s) to within ~3%.  ACT computes exact exp via scale=1/C1.
C1 = 128.0 / float(np.log(2.0))
C2 = 127.0 * 128.0 - 5.5
ACT_SLOTS = 5                      # slots 0-4 exact exp on ACT
ACT_COLS = ACT_SLOTS * 128         # 640; DVE takes cols 640:1152

F32 = mybir.dt.float32
F16 = mybir.dt.float16
I16 = mybir.dt.int16
BF16 = mybir.dt.bfloat16

# slot/band layout of the 9 score tiles inside one [128, 1152] fp32 PSUM
# half.  Concurrent row-tiled matmuls must land in different 2KB PSUM
# banks (slot s lives in bank s//4).  The first three tiles run as a
# 3-way concurrent group on row bands 0/64/32 (k and q are replicated on
# partitions 0-15, 32-47, 64-79) hitting banks 0/1/2; the remaining six
# run as 2-way pairs on bands 0/64 hitting banks 0/1.
_SLOT = {0: 0, 1: 4, 2: 8, 3: 1, 4: 5, 5: 2, 6: 6, 7: 3, 8: 7}
_BAND = {0: 0, 1: 64, 2: 32, 3: 0, 4: 64, 5: 0, 6: 64, 7: 0, 8: 64}


def _build_nc(kw_fold=True):
    # kw_fold (valid when bq == 0, which the harness always supplies):
    # fold the q-projection into the score matmuls via KWT = C1*Wq@k^T
    # computed on-device.  Scores become full-K (K=128) matmuls: unlike
    # the K=16 row-group-masked form, these register as PE activity in
    # the HAM clock gate, so the steady loop can un-throttle to 2.4 GHz.
    # Also removes the q projection + its 18 ACT evacuations.
    nc = bass.Bass(target_bir_lowering=False)

    x32 = nc.dram_tensor("x32", [N, C], F32, kind="ExternalInput")
    # x transposed [C, N] on the HOST: a plain DMA loads it (Tile
    # serializes every xbar-transpose DMA against all other in-flight
    # DMAs -- the HW-deadlock guard -- which stretched the 6 transposed
    # xT chunk loads across ~40us of the preamble).
    xt_d = nc.dram_tensor("xt", [C, N], BF16, kind="ExternalInput")
    wv_d = nc.dram_tensor("wv", [C, C], BF16, kind="ExternalInput")
    if kw_fold:
        wqt_d = nc.dram_tensor("wqt", [DK, C], BF16, kind="ExternalInput")
    else:
        wq2_d = nc.dram_tensor("wq2", [C, 80], BF16, kind="ExternalInput")
        bq2_d = nc.dram_tensor("bq2", [80, 1], F32, kind="ExternalInput")
    wk2_d = nc.dram_tensor("wk2", [C, 80], BF16, kind="ExternalInput")
    beta_d = nc.dram_tensor("beta128", [C, 1], F32, kind="ExternalInput")
    out_d = nc.dram_tensor("out", [N, C], F32, kind="ExternalOutput")

    # exp engine split: with full-K scores the PE runs warm and ACT
    # becomes the pacer, so give DVE a bigger share (512/640 vs 640/512).
    # exp split: at 512 the warm-loop trace shows DVE 102% busy vs ACT
    # 73% -- shift exp columns to ACT until both pace equally (~1.03us).
    act_cols = 704 if kw_fold else ACT_COLS

    dma = nc.default_dma_engine

    with TileContext(nc) as tc:
        with (
            tc.tile_pool(name="const", bufs=1) as cpool,
            tc.tile_pool(name="big", bufs=1) as big,
        ):
            wv_sb = cpool.tile([C, C], BF16)
            wk2_sb = cpool.tile([C, 80], BF16)
            beta_sb = cpool.tile([C, 1], F32)
            loads = [(wv_sb, wv_d), (wk2_sb, wk2_d), (beta_sb, beta_d)]
            if kw_fold:
                wqt_sb = cpool.tile([DK, C], BF16)
                loads.append((wqt_sb, wqt_d))
            else:
                wq2_sb = cpool.tile([C, 80], BF16)
                bq2_sb = cpool.tile([80, 1], F32)
                loads += [(wq2_sb, wq2_d), (bq2_sb, bq2_d)]
            for sb, d in loads:
                dma.dma_start(sb[:, :], d[:, :])

            x_nat = big.tile([128, NT, 128], F32)
            xT = big.tile([128, N], BF16)
            if kw_fold:
                kwt_sb = big.tile([128, NP], BF16)
            else:
                qT = big.tile([128, N], F16)
            kpool = big.tile([128, NP], F16)
            vpoolT = big.tile([128, NP], BF16)
            vaug = big.tile([128, KT, VSLOT], BF16)

            # xT first (it gates every projection matmul), chunked so
            # the first k matmul can start early.
            for c6 in range(6):
                dma.dma_start(
                    xT[:, c6 * 1536:(c6 + 1) * 1536],
                    xt_d[:, c6 * 1536:(c6 + 1) * 1536],
                )
            # x (natural fp32, with beta*bv folded in host-side) is only
            # read by the residual adds at the end of each q tile.
            for c6 in range(6):
                dma.dma_start(
                    x_nat[:, c6 * 12:(c6 + 1) * 12, :],
                    x32[c6 * 12 * 128:(c6 + 1) * 12 * 128, :].rearrange(
                        "(t p) c -> p t c", p=128
                    ),
                )

            # ones column for the softmax-denominator trick; transposed V
            # tiles later overwrite cols 0-127 of each slot.
            nc.vector.memset(vaug[:, :, :], 1.0)

            # ---- projections + pooling --------------------------------
            # PE: k, v, q matmuls.  DVE: maxpool reductions straight from
            # PSUM.  ACT: q evacuation with bias folded.  DMA: transposes
            # pooled V into vaug [kpix, C] slots.
            # bk needs no add at all: softmax(q.(k+bk)) == softmax(q.k)
            # (a per-row constant shift).  bv is folded into the residual
            # host-side (out = (x + beta*bv) + beta*(o/d)).
            with (
                tc.tile_pool(name="ppsum", bufs=3, space="PSUM") as ppool,
                tc.tile_pool(name="qpsum", bufs=3, space="PSUM") as qpool,
            ):
                for ch in range(HP):
                    pk = ppool.tile([128, 2 * W], F32, tag="pp")
                    nc.tensor.matmul(
                        pk[0:80, :], wk2_sb[:, :],
                        xT[:, ch * 2 * W:(ch + 1) * 2 * W],
                        start=True, stop=True,
                    )
                    nc.vector.tensor_reduce(
                        out=kpool[0:80, ch * WP:(ch + 1) * WP],
                        in_=pk[0:80, :].rearrange(
                            "p (hh w2 wp) -> p w2 hh wp", hh=2, wp=2
                        ),
                        axis=mybir.AxisListType.XY,
                        op=mybir.AluOpType.max,
                    )

                for ch in range(HP):
                    pv = ppool.tile([128, 2 * W], F32, tag="pp")
                    nc.tensor.matmul(
                        pv[:, :], wv_sb[:, :],
                        xT[:, ch * 2 * W:(ch + 1) * 2 * W],
                        start=True, stop=True,
                    )
                    nc.vector.tensor_reduce(
                        out=vpoolT[:, ch * WP:(ch + 1) * WP],
                        in_=pv[:, :].rearrange(
                            "p (hh w2 wp) -> p w2 hh wp", hh=2, wp=2
                        ),
                        axis=mybir.AxisListType.XY,
                        op=mybir.AluOpType.max,
                    )

                if kw_fold:
                    # KWT[c, kpix] = C1 * Wq @ k^T, evacuated bf16; the
                    # scores are then KWT_t^T @ xT with K=128.
                    for off in range(0, NP, 512):
                        w = min(512, NP - off)
                        pq = qpool.tile([128, 512], F32, tag="pq")
                        nc.tensor.matmul(
                            pq[:, 0:w], wqt_sb[:, :],
                            kpool[0:DK, off:off + w],
                            start=True, stop=True,
                        )
                        nc.scalar.activation(
                            out=kwt_sb[:, off:off + w], in_=pq[:, 0:w],
                            func=mybir.ActivationFunctionType.Identity,
                        )
                else:
                    for ch in range(KT):
                        pq = qpool.tile([128, 512], F32, tag="pq")
                        nc.tensor.matmul(
                            pq[0:80, :], wq2_sb[:, :],
                            xT[:, ch * 512:(ch + 1) * 512],
                            start=True, stop=True,
                        )
                        nc.scalar.activation(
                            out=qT[0:80, ch * 512:(ch + 1) * 512],
                            in_=pq[0:80, :],
                            func=mybir.ActivationFunctionType.Identity,
                            bias=bq2_sb[:, 0:1],
                        )

                # transpose pooled V into [kpix, C] tiles on the DMA engine
                for t in range(KT):
                    dma.dma_start_transpose(
                        vaug[:, t, 0:128],
                        vpoolT[:, t * 128:(t + 1) * 128],
                    )

            # ---- attention main loop ----------------------------------
            # Software-pipelined with 2-half lookahead: the PE stream is
            # s(0) s(1) s(2) pv(0) s(3) pv(1) ... with no dependency
            # stalls, so the HAM clock gate keeps the PE at 2.4 GHz.
            # exp is ACT-only for the first halves (DVE is still draining
            # the preamble maxpool reductions), then splits ACT/DVE.
            # finish: DVE computes (o * 1/d) * beta, GpSimd adds the
            # residual (x already carries beta*bv), keeping DVE light.
            ACT_ONLY = 8
            with (
                tc.tile_pool(name="spsum", bufs=3, space="PSUM") as spool,
                tc.tile_pool(name="spsum1", bufs=1, space="PSUM") as spool1,
                tc.tile_pool(name="opsum", bufs=1, space="PSUM") as opool,
                tc.tile_pool(name="expp", bufs=4) as epool,
                tc.tile_pool(name="outp", bufs=4) as outpool,
                tc.tile_pool(name="ob1p", bufs=4) as ob1pool,
                tc.tile_pool(name="sclp", bufs=4) as sclpool,
            ):
                sing = spool1.tile([128, 3 * 128], F32)

                def scores(h):
                    q, half = divmod(h, 2)
                    s_ps = spool.tile([128, 8 * 128], F32, tag="s")
                    for tt in range(9):
                        t = half * 9 + tt
                        slot, band = _SLOT[tt], _BAND[tt]
                        if slot == 8:
                            dst = sing[:, (h % 3) * 128:(h % 3 + 1) * 128]
                        else:
                            dst = s_ps[:, slot * 128:(slot + 1) * 128]
                        if kw_fold:
                            nc.tensor.matmul(
                                dst,
                                kwt_sb[:, t * 128:(t + 1) * 128],
                                xT[:, q * 128:(q + 1) * 128],
                                start=True, stop=True,
                            )
                        else:
                            nc.tensor.matmul(
                                dst,
                                kpool[band:band + 16, t * 128:(t + 1) * 128],
                                qT[band:band + 16, q * 128:(q + 1) * 128],
                                start=True, stop=True,
                                tile_position=(band, 0),
                            )
                    return s_ps

                def exp_half(h, s_ps):
                    # high_priority: when an exp and a finish op are both
                    # ready on ACT/DVE, schedule the exp first -- a finish
                    # op ahead of exp in the FIFO stalls the PE's scores.
                    ctx = tc.high_priority()
                    ctx.__enter__()
                    e = epool.tile([128, 9 * 128], BF16, tag="exp")
                    ss = sing[:, (h % 3) * 128:(h % 3 + 1) * 128]
                    if h < ACT_ONLY:
                        nc.scalar.activation(
                            out=e[:, 0:1024], in_=s_ps[:, 0:1024],
                            func=mybir.ActivationFunctionType.Exp,
                            scale=1.0 / C1,
                        )
                        nc.scalar.activation(
                            out=e[:, 1024:1152], in_=ss,
                            func=mybir.ActivationFunctionType.Exp,
                            scale=1.0 / C1,
                        )
                    else:
                        nc.scalar.activation(
                            out=e[:, 0:act_cols], in_=s_ps[:, 0:act_cols],
                            func=mybir.ActivationFunctionType.Exp,
                            scale=1.0 / C1,
                        )
                        nc.vector.tensor_scalar(
                            out=e[:, act_cols:1024].bitcast(I16),
                            in0=s_ps[:, act_cols:1024],
                            scalar1=C2, scalar2=None,
                            op0=mybir.AluOpType.add,
                        )
                        nc.vector.tensor_scalar(
                            out=e[:, 1024:1152].bitcast(I16),
                            in0=ss,
                            scalar1=C2, scalar2=None,
                            op0=mybir.AluOpType.add,
                        )
                    ctx.__exit__(None, None, None)
                    return e

                def pv(h, e, o_ps):
                    half = h % 2
                    for tt in range(9):
                        t = half * 9 + tt
                        slot = _SLOT[tt]
                        nc.tensor.matmul(
                            o_ps[:, 0:129],
                            e[:, slot * 128:(slot + 1) * 128],
                            vaug[:, t, 0:129],
                            start=(t == 0), stop=(t == KT - 1),
                        )

                def finish(q, o_ps):
                    # ACT (which trails the PV by design) evacuates o to
                    # SBUF so no DVE op ever waits on PV completion --
                    # a PV-dependent op at the DVE FIFO head would starve
                    # the next half's exp and stall the PE.
                    osb = ob1pool.tile([128, 129], F32, tag="osb")
                    nc.scalar.activation(
                        out=osb[:, :], in_=o_ps[:, 0:129],
                        func=mybir.ActivationFunctionType.Copy,
                    )
                    # normalize on DVE (reads SBUF only, so it never waits
                    # on PV at the DVE FIFO head); residual add on GpSimd.
                    recip = sclpool.tile([128, 1], F32, tag="recip")
                    nc.vector.reciprocal(recip[:, :], osb[:, 128:129])
                    ob1 = ob1pool.tile([128, 128], F32, tag="ob1")
                    nc.vector.tensor_scalar(
                        out=ob1[:, :], in0=osb[:, 0:128],
                        scalar1=recip[:, 0:1], scalar2=beta_sb[:, 0:1],
                        op0=mybir.AluOpType.mult, op1=mybir.AluOpType.mult,
                    )
                    ob = outpool.tile([128, 128], F32, tag="ob")
                    nc.gpsimd.tensor_tensor(
                        out=ob[:, :], in0=ob1[:, :], in1=x_nat[:, q, :],
                        op=mybir.AluOpType.add,
                    )
                    dma.dma_start(out_d[q * 128:(q + 1) * 128, :], ob[:, :])

                # finish(q) is emitted two halves AFTER pv(2q+1) so its
                # pv-completion wait never head-of-line-blocks the next
                # half's DVE exp (strict 8-deep FIFO).
                es = []
                o_tiles = {}
                for h in range(NHALF + 4):
                    if h < NHALF:
                        s_ps = scores(h)
                        es.append(exp_half(h, s_ps))
                    hp = h - 2
                    if 0 <= hp < NHALF:
                        if hp % 2 == 0:
                            o_tiles[hp // 2] = opool.tile(
                                [128, 129], F32, tag="o", name="o_ps"
                            )
                        pv(hp, es.pop(0), o_tiles[hp // 2])
                    if h >= 5 and (h - 5) % 2 == 0:
                        q = (h - 5) // 2
                        finish(q, o_tiles.pop(q))

    return nc


_CACHE = {}


def _get_nc(kw_fold):
    key = f"nc{int(kw_fold)}"
    if key not in _CACHE:
        _CACHE[key] = _build_nc(kw_fold)
    return _CACHE[key]


def kernel(query, Wq, bq, Wk, bk, Wv, bv, beta, _trace=False):
    query = np.ascontiguousarray(np.asarray(query, dtype=np.float32))
    Wq = np.asarray(Wq, dtype=np.float32)
    Wk = np.asarray(Wk, dtype=np.float32)
    Wv = np.asarray(Wv, dtype=np.float32)
    bq = np.asarray(bq, dtype=np.float32)
    bk = np.asarray(bk, dtype=np.float32)
    bv = np.asarray(bv, dtype=np.float32)
    beta = np.asarray(beta, dtype=np.float32)

    # replicate q/k weights into PE columns 0-15, 32-47 and 64-79 so
    # their projections land on the row-tiled sub-arrays' SBUF
    # partitions (bands 0/32/64 for the 3-way score group).  Wq/bq are
    # additionally scaled by C1 so the scores arrive pre-scaled for the
    # Schraudolph exp (see module docstring).
    wq2 = np.zeros((C, 80), dtype=np.float32)
    wk2 = np.zeros((C, 80), dtype=np.float32)
    bq2 = np.zeros((80, 1), dtype=np.float32)
    for off in (0, 32, 64):
        wq2[:, off:off + DK] = Wq * C1
        wk2[:, off:off + DK] = Wk
        bq2[off:off + DK, 0] = bq * C1
    # bk never needs to be applied: softmax(q.(k+bk)) == softmax(q.k)
    # (it shifts every logit in a row by the same q.bk constant).

    beta_s = float(beta.ravel()[0])
    # bq == 0 (always, for this harness) enables the kw_fold graph: the
    # q projection folds into the score matmuls via KWT = C1*Wq@k^T.
    kw_fold = not np.any(bq)
    shared = {
        "wv": Wv.astype(ml_dtypes.bfloat16),
        "wk2": wk2.astype(ml_dtypes.bfloat16),
        "beta128": np.full((C, 1), beta_s, dtype=np.float32),
    }
    if kw_fold:
        shared["wqt"] = np.ascontiguousarray(
            (Wq * C1).T.astype(ml_dtypes.bfloat16)
        )
    else:
        shared["wq2"] = wq2.astype(ml_dtypes.bfloat16)
        shared["bq2"] = bq2
    # fold beta*bv into the residual: out = (x + beta*bv) + beta*(o/d)
    bv_fold = (beta_s * bv).astype(np.float32)[None, :]
    in_maps = []
    for b in range(B):
        img = query[b].reshape(N, C)
        m = dict(shared)
        m["x32"] = img + bv_fold
        m["xt"] = np.ascontiguousarray(img.T.astype(ml_dtypes.bfloat16))
        in_maps.append(m)

    nc = _get_nc(kw_fold)
    res = run_bass_kernel_spmd(
        nc, in_maps, core_ids=list(range(B)), trace=_trace
    )
    out = np.stack([res.results[b]["out"] for b in range(B)])
    out = out.reshape(B, H, W, C).astype(np.float32)
    if _trace:
        kernel.last_exec_time_ns = res.exec_time_ns
        kernel.last_profile_json = res.profile_json
    return out



# revision 49
# speedup vs baseline: 1.3930x; 1.0699x over previous
"""Trainium2 Bass kernel for ConditionalAttentionConv2D.

Reference computation (per image, B=8 images total):
    k = maxpool2x2(x @ Wk + bk)          [2304, 16]
    q = x @ Wq + bq                      [9216, 16]
    s = softmax(q @ k^T, axis=-1)        [9216, 2304]
    v = maxpool2x2(x @ Wv + bv)          [2304, 128]
    out = x + beta * (s @ v)             [9216, 128]

Sharding: data-parallel over batch, one image per NeuronCore (8 cores).

Key performance structure:
  - The attention loop is software-pipelined with a 2-half lookahead
    (scores run two half-tiles ahead of the PV matmuls) to minimize PE
    dependency stalls (the HAM clock gate throttles the PE to 1.2 GHz
    whenever its activity window sees idle gaps).
  - exp is split across two engines so it keeps pace with the PE: ACT
    computes exact exp on slots 0-4 (with scale=1/C1), DVE computes a
    Schraudolph-style exp on slots 5-8: scores arrive pre-scaled by
    C1 = 2^7*log2(e) (folded into Wq on the host), DVE adds the exponent
    bias C2 and converts fp32->int16, whose bits ARE the bf16 exp value.
    The softmax denominator (ones column appended to V) sums the actual
    e values used, so rows still normalize exactly.  The first halves are
    ACT-only while DVE drains the preamble maxpool reductions.
  - The first three score tiles of each half run as a 3-way concurrent
    row-tiled group (bands 0/64/32, PSUM banks 0/1/2); the rest as 2-way
    pairs.  k/q are replicated on partitions 0-15, 32-47, 64-79.
  - bk is never applied (softmax is invariant to the per-row constant
    q.bk); beta*bv is folded into the residual input host-side.
  - finish: ACT evacuates o from PSUM, DVE normalizes from SBUF, GpSimd
    adds the residual -- no DVE op ever waits on PV completion, which
    would head-of-line-block the next half's exp in the DVE FIFO.
  - Walrus's LDW optimization is enabled by re-merging tile_legalize's
    standalone Ldweights into their Matmults at the BIR-JSON level (see
    _merge_ldweights).
  - Preamble: xT is transposed on the HOST and loaded by plain DMA
    chunks (Tile serializes every xbar-transpose DMA against all other
    in-flight DMAs -- the HW-deadlock guard -- so transposed loads
    stretched across ~40us of the preamble); pooled-V tiles are
    transposed into vaug by the DMA engine (slot stride 160: the XBAR
    transpose DMA corrupts non-32-aligned destination offsets).
"""

import os
import numpy as np

import ml_dtypes

from concourse import bass, mybir, masks
from concourse.tile import TileContext
from concourse.bass_utils import run_bass_kernel_spmd

# ---------------------------------------------------------------------------
# Walrus in this toolchain rejects >1 sync-wait on a CTRL instruction, but
# TileContext's final drain carries one wait per active proc.  Split them
# across standalone sync-engine NOPs before a bare drain.
# ---------------------------------------------------------------------------


def _patched_drain_and_barrier(self, tick_clock, wait_clock):
    from concourse.vector_clock import ScopedClock

    nop_inst = self.nc.sync.nop(nofuse=True)
    wait_clock.add_sem_waits(
        nop_inst.ins, ScopedClock({None: tick_clock.global_clock})
    )
    si = nop_inst.ins.sync_info
    waits = list(si.on_wait) if si is not None else []
    if len(waits) > 1:
        del si.on_wait[1:]
        for w in waits[1:]:
            n2 = self.nc.sync.nop(nofuse=True)
            n2.ins.sync_info = mybir.SyncInfo(on_wait=[w], on_update=[])
    self.nc.sync.drain()
    self.nc.all_engine_barrier()
    popped = self.nc._tile_sem_poison_stack.pop()
    assert popped is self._sem_poison
    self.nc.clear_and_free_semaphores(list(self.sems.allocated().values()))
    self.nc.all_engine_barrier()


TileContext._drain_and_barrier = _patched_drain_and_barrier


def _tile_structural_classes():
    import concourse.tile as _t

    names = (
        "BassTileCriticalSection", "BassTileConditionalBlock",
        "TileBranchInst", "BassTileRelease",
        "BassTileBranchHintPlaceholder", "BassTileLoopBlock",
    )
    return tuple(getattr(_t, n) for n in names if hasattr(_t, n))


_STRUCTURAL = None
_orig_commit_and_lower = TileContext._commit_and_lower


def _patched_commit_and_lower(self, inst, original_block, old_bb_map,
                              bb_to_exit_bb):
    global _STRUCTURAL
    if _STRUCTURAL is None:
        _STRUCTURAL = _tile_structural_classes()
    si = getattr(inst, "sync_info", None)
    # Ldweights cannot carry waits under walrus's LDW optimization, and no
    # instruction can carry more than one wait on this toolchain: hoist the
    # excess onto same-engine NOPs committed immediately before.
    keep = 0 if isinstance(inst, mybir.InstLdweights) else 1
    if (
        si is not None
        and si.on_wait
        and len(si.on_wait) > keep
        and not isinstance(inst, _STRUCTURAL)
    ):
        waits = list(si.on_wait)
        si.on_wait[:] = waits[len(waits) - keep:]
        for i, w in enumerate(waits[:len(waits) - keep]):
            nop = mybir.InstNoOp(
                name=f"{inst.name}-sw{i}",
                engine=inst.engine,
                sync_info=mybir.SyncInfo(on_wait=[w], on_update=[]),
                bass_nofuse=True,
            )
            self._commit_instruction(nop)
    return _orig_commit_and_lower(
        self, inst, original_block, old_bb_map, bb_to_exit_bb
    )


TileContext._commit_and_lower = _patched_commit_and_lower

# ---------------------------------------------------------------------------
# Enable walrus's LDW optimization (incl. Fast Weight Load: 2x bf16
# weight-load bandwidth).  tile_legalize always splits InstMatmult into a
# standalone InstLdweights + a non-self-loading InstMatmult, which walrus
# rejects under --enable-ldw-opt.  So just before walrus runs, merge each
# Ldweights back into its Matmult (ldweights=true) at the BIR-JSON level
# and flip the flag.  The TileContext commit patch above already keeps
# sem-waits OFF the Ldweights instructions (hoisted to NOPs), so the merge
# only has to migrate on_update entries.
# ---------------------------------------------------------------------------

import json as _json

from concourse import bass_utils as _bu
from concourse import hw_specs as _hw

# ---------------------------------------------------------------------------
# Make the Tile scheduler's cost model see the PE at its HAM-throttled
# 1.2 GHz rate.  The default 2.4 GHz model makes the simulated PV matmuls
# finish early, so the scheduler places PV-dependent ops (o-evacuation,
# normalize) ahead of the next half's exp in the ACT/DVE FIFOs; on real
# (cold) hardware those ops then wait at the FIFO head and stall the PE.
# This only changes instruction scheduling, not hardware behavior.
# ---------------------------------------------------------------------------
_hw.TRN2Spec.PE_CYCLE = 1e9 / 1.2e9


def _merge_ldweights(bir_json: bytes) -> bytes:
    bir = _json.loads(bir_json)
    for fn in bir["functions"]:
        for blk in fn["blocks"]:
            out = []
            pending = None
            for inst in blk["instructions"]:
                op = inst.get("opcode")
                if op == "Ldweights":
                    assert pending is None, "two Ldweights without Matmult"
                    si = inst.get("sync_info") or {}
                    assert not si.get("on_wait"), (
                        f"LDW {inst['name']} carries waits"
                    )
                    pending = inst
                    continue
                if op == "Matmult" and pending is not None:
                    w = pending["ins"][0]
                    mw = inst["ins"][1]
                    assert (
                        w["memref"] == mw["memref"]
                        and w["offset"] == mw["offset"]
                    ), f"LDW/MM pairing mismatch {pending['name']} {inst['name']}"
                    inst["ldweights"] = True
                    lu = (pending.get("sync_info") or {}).get("on_update") or []
                    if lu:
                        inst["sync_info"]["on_update"].extend(lu)
                    pending = None
                out.append(inst)
            assert pending is None, "trailing Ldweights"
            blk["instructions"] = out
    return _json.dumps(bir).encode()


_orig_run_command = _bu.run_command


def _patched_run_command(argv, **kwargs):
    if isinstance(argv, list):
        argv = [
            "--enable-ldw-opt=true" if a == "--enable-ldw-opt=false" else a
            for a in argv
        ]
    return _orig_run_command(argv, **kwargs)


_bu.run_command = _patched_run_command

_orig_compile_impl = _bu._compile_bir_impl


def _patched_compile_impl(bir_json, *args, **kwargs):
    return _orig_compile_impl(_merge_ldweights(bir_json), *args, **kwargs)


_bu._compile_bir_impl = _patched_compile_impl

# ---------------------------------------------------------------------------

B, H, W, C = 8, 48, 192, 128
DK = C // 8               # 16
N = H * W                 # 9216 pixels
NT = N // 128             # 72 q tiles
NHALF = 2 * NT            # 144 half-tiles of 9 kpix slots each
KT = 18                   # pooled-pixel tiles of 128
NP = KT * 128             # 2304 pooled pixels
HP, WP = H // 2, W // 2   # 24, 96
VSLOT = 160               # vpool slot stride: 128 ch + 1 ones + pad
                          # (32-aligned: the XBAR transpose DMA corrupts
                          # non-32-aligned destination column offsets)

# Schraudolph exp: scores arrive pre-scaled by C1 = 2^7/ln2 (folded into
# Wq host-side).  DVE computes int16(s' + C2); those bits read as bf16 give
# exp(s) to within ~3%.  ACT computes exact exp via scale=1/C1.
C1 = 128.0 / float(np.log(2.0))
C2 = 127.0 * 128.0 - 5.5
ACT_SLOTS = 5                      # slots 0-4 exact exp on ACT
ACT_COLS = ACT_SLOTS * 128         # 640; DVE takes cols 640:1152

F32 = mybir.dt.float32
F16 = mybir.dt.float16
I16 = mybir.dt.int16
BF16 = mybir.dt.bfloat16

# slot/band layout of the 9 score tiles inside one [128, 1152] fp32 PSUM
# half.  Concurrent row-tiled matmuls must land in different 2KB PSUM
# banks (slot s lives in bank s//4).  The first three tiles run as a
# 3-way concurrent group on row bands 0/64/32 (k and q are replicated on
# partitions 0-15, 32-47, 64-79) hitting banks 0/1/2; the remaining six
# run as 2-way pairs on bands 0/64 hitting banks 0/1.
_SLOT = {0: 0, 1: 4, 2: 8, 3: 1, 4: 5, 5: 2, 6: 6, 7: 3, 8: 7}
_BAND = {0: 0, 1: 64, 2: 32, 3: 0, 4: 64, 5: 0, 6: 64, 7: 0, 8: 64}


def _build_nc(kw_fold=True):
    # kw_fold (valid when bq == 0, which the harness always supplies):
    # fold the q-projection into the score matmuls via KWT = C1*Wq@k^T
    # computed on-device.  Scores become full-K (K=128) matmuls: unlike
    # the K=16 row-group-masked form, these register as PE activity in
    # the HAM clock gate, so the steady loop can un-throttle to 2.4 GHz.
    # Also removes the q projection + its 18 ACT evacuations.
    nc = bass.Bass(target_bir_lowering=False)

    x32 = nc.dram_tensor("x32", [N, C], F32, kind="ExternalInput")
    # x transposed [C, N] on the HOST: a plain DMA loads it (Tile
    # serializes every xbar-transpose DMA against all other in-flight
    # DMAs -- the HW-deadlock guard -- which stretched the 6 transposed
    # xT chunk loads across ~40us of the preamble).
    xt_d = nc.dram_tensor("xt", [C, N], BF16, kind="ExternalInput")
    wv_d = nc.dram_tensor("wv", [C, C], BF16, kind="ExternalInput")
    if kw_fold:
        wqt_d = nc.dram_tensor("wqt", [DK, C], BF16, kind="ExternalInput")
    else:
        wq2_d = nc.dram_tensor("wq2", [C, 80], BF16, kind="ExternalInput")
        bq2_d = nc.dram_tensor("bq2", [80, 1], F32, kind="ExternalInput")
    wk2_d = nc.dram_tensor("wk2", [C, 80], BF16, kind="ExternalInput")
    beta_d = nc.dram_tensor("beta128", [C, 1], F32, kind="ExternalInput")
    out_d = nc.dram_tensor("out", [N, C], F32, kind="ExternalOutput")

    # exp engine split: with full-K scores the PE runs warm and ACT
    # becomes the pacer, so give DVE a bigger share (512/640 vs 640/512).
    act_cols = 512 if kw_fold else ACT_COLS

    dma = nc.default_dma_engine

    with TileContext(nc) as tc:
        with (
            tc.tile_pool(name="const", bufs=1) as cpool,
            tc.tile_pool(name="big", bufs=1) as big,
        ):
            wv_sb = cpool.tile([C, C], BF16)
            wk2_sb = cpool.tile([C, 80], BF16)
            beta_sb = cpool.tile([C, 1], F32)
            loads = [(wv_sb, wv_d), (wk2_sb, wk2_d), (beta_sb, beta_d)]
            if kw_fold:
                wqt_sb = cpool.tile([DK, C], BF16)
                loads.append((wqt_sb, wqt_d))
            else:
                wq2_sb = cpool.tile([C, 80], BF16)
                bq2_sb = cpool.tile([80, 1], F32)
                loads += [(wq2_sb, wq2_d), (bq2_sb, bq2_d)]
            for sb, d in loads:
                dma.dma_start(sb[:, :], d[:, :])

            x_nat = big.tile([128, NT, 128], F32)
            xT = big.tile([128, N], BF16)
            if kw_fold:
                kwt_sb = big.tile([128, NP], BF16)
            else:
                qT = big.tile([128, N], F16)
            kpool = big.tile([128, NP], F16)
            vpoolT = big.tile([128, NP], BF16)
            vaug = big.tile([128, KT, VSLOT], BF16)

            # xT first (it gates every projection matmul), chunked so
            # the first k matmul can start early.
            for c6 in range(6):
                dma.dma_start(
                    xT[:, c6 * 1536:(c6 + 1) * 1536],
                    xt_d[:, c6 * 1536:(c6 + 1) * 1536],
                )
            # x (natural fp32, with beta*bv folded in host-side) is only
            # read by the residual adds at the end of each q tile.
            for c6 in range(6):
                dma.dma_start(
                    x_nat[:, c6 * 12:(c6 + 1) * 12, :],
                    x32[c6 * 12 * 128:(c6 + 1) * 12 * 128, :].rearrange(
                        "(t p) c -> p t c", p=128
                    ),
                )

            # ones column for the softmax-denominator trick; transposed V
            # tiles later overwrite cols 0-127 of each slot.
            nc.vector.memset(vaug[:, :, :], 1.0)
            if kw_fold:
                ident = big.tile([128, 128], BF16)
                masks.make_identity(nc, ident[:, :])

            # ---- projections + pooling --------------------------------
            # PE: k, v, q matmuls.  DVE: maxpool reductions straight from
            # PSUM.  ACT: q evacuation with bias folded.  DMA: transposes
            # pooled V into vaug [kpix, C] slots.
            # bk needs no add at all: softmax(q.(k+bk)) == softmax(q.k)
            # (a per-row constant shift).  bv is folded into the residual
            # host-side (out = (x + beta*bv) + beta*(o/d)).
            with (
                tc.tile_pool(name="ppsum", bufs=3, space="PSUM") as ppool,
                tc.tile_pool(name="qpsum", bufs=3, space="PSUM") as qpool,
            ):
                for ch in range(HP):
                    pk = ppool.tile([128, 2 * W], F32, tag="pp")
                    nc.tensor.matmul(
                        pk[0:80, :], wk2_sb[:, :],
                        xT[:, ch * 2 * W:(ch + 1) * 2 * W],
                        start=True, stop=True,
                    )
                    nc.vector.tensor_reduce(
                        out=kpool[0:80, ch * WP:(ch + 1) * WP],
                        in_=pk[0:80, :].rearrange(
                            "p (hh w2 wp) -> p w2 hh wp", hh=2, wp=2
                        ),
                        axis=mybir.AxisListType.XY,
                        op=mybir.AluOpType.max,
                    )

                for ch in range(HP):
                    pv = ppool.tile([128, 2 * W], F32, tag="pp")
                    nc.tensor.matmul(
                        pv[:, :], wv_sb[:, :],
                        xT[:, ch * 2 * W:(ch + 1) * 2 * W],
                        start=True, stop=True,
                    )
                    nc.vector.tensor_reduce(
                        out=vpoolT[:, ch * WP:(ch + 1) * WP],
                        in_=pv[:, :].rearrange(
                            "p (hh w2 wp) -> p w2 hh wp", hh=2, wp=2
                        ),
                        axis=mybir.AxisListType.XY,
                        op=mybir.AluOpType.max,
                    )

                if kw_fold:
                    # KWT[c, kpix] = C1 * Wq @ k^T, evacuated bf16; the
                    # scores are then KWT_t^T @ xT with K=128.
                    for off in range(0, NP, 512):
                        w = min(512, NP - off)
                        pq = qpool.tile([128, 512], F32, tag="pq")
                        nc.tensor.matmul(
                            pq[:, 0:w], wqt_sb[:, :],
                            kpool[0:DK, off:off + w],
                            start=True, stop=True,
                        )
                        nc.scalar.activation(
                            out=kwt_sb[:, off:off + w], in_=pq[:, 0:w],
                            func=mybir.ActivationFunctionType.Identity,
                        )
                else:
                    for ch in range(KT):
                        pq = qpool.tile([128, 512], F32, tag="pq")
                        nc.tensor.matmul(
                            pq[0:80, :], wq2_sb[:, :],
                            xT[:, ch * 512:(ch + 1) * 512],
                            start=True, stop=True,
                        )
                        nc.scalar.activation(
                            out=qT[0:80, ch * 512:(ch + 1) * 512],
                            in_=pq[0:80, :],
                            func=mybir.ActivationFunctionType.Identity,
                            bias=bq2_sb[:, 0:1],
                        )

                if kw_fold:
                    # transpose pooled V into [kpix, C] tiles on the PE
                    # (identity matmul) with DVE evacuation: the 18
                    # xbar-transpose DMAs were serialized against each
                    # other by the HW-deadlock guard (~23us), gating
                    # PV(0) and stretching the ramp to ~60us.
                    for t in range(KT):
                        tps = ppool.tile(
                            [128, 128], BF16, tag="pp", name="tps"
                        )
                        nc.tensor.transpose(
                            tps[:, :],
                            vpoolT[:, t * 128:(t + 1) * 128],
                            ident[:, :],
                        )
                        nc.vector.tensor_copy(vaug[:, t, 0:128], tps[:, :])
                else:
                    # transpose pooled V into [kpix, C] tiles on the DMA
                    for t in range(KT):
                        dma.dma_start_transpose(
                            vaug[:, t, 0:128],
                            vpoolT[:, t * 128:(t + 1) * 128],
                        )

            # ---- attention main loop ----------------------------------
            # Software-pipelined with 2-half lookahead: the PE stream is
            # s(0) s(1) s(2) pv(0) s(3) pv(1) ... with no dependency
            # stalls, so the HAM clock gate keeps the PE at 2.4 GHz.
            # exp is ACT-only for the first halves (DVE is still draining
            # the preamble maxpool reductions), then splits ACT/DVE.
            # finish: DVE computes (o * 1/d) * beta, GpSimd adds the
            # residual (x already carries beta*bv), keeping DVE light.
            ACT_ONLY = 8
            with (
                tc.tile_pool(name="spsum", bufs=3, space="PSUM") as spool,
                tc.tile_pool(name="spsum1", bufs=1, space="PSUM") as spool1,
                tc.tile_pool(name="opsum", bufs=1, space="PSUM") as opool,
                tc.tile_pool(name="expp", bufs=4) as epool,
                tc.tile_pool(name="outp", bufs=4) as outpool,
                tc.tile_pool(name="ob1p", bufs=4) as ob1pool,
                tc.tile_pool(name="sclp", bufs=4) as sclpool,
            ):
                sing = spool1.tile([128, 3 * 128], F32)

                def scores(h):
                    q, half = divmod(h, 2)
                    s_ps = spool.tile([128, 8 * 128], F32, tag="s")
                    for tt in range(9):
                        t = half * 9 + tt
                        slot, band = _SLOT[tt], _BAND[tt]
                        if slot == 8:
                            dst = sing[:, (h % 3) * 128:(h % 3 + 1) * 128]
                        else:
                            dst = s_ps[:, slot * 128:(slot + 1) * 128]
                        if kw_fold:
                            nc.tensor.matmul(
                                dst,
                                kwt_sb[:, t * 128:(t + 1) * 128],
                                xT[:, q * 128:(q + 1) * 128],
                                start=True, stop=True,
                            )
                        else:
                            nc.tensor.matmul(
                                dst,
                                kpool[band:band + 16, t * 128:(t + 1) * 128],
                                qT[band:band + 16, q * 128:(q + 1) * 128],
                                start=True, stop=True,
                                tile_position=(band, 0),
                            )
                    return s_ps

                def exp_half(h, s_ps):
                    # high_priority: when an exp and a finish op are both
                    # ready on ACT/DVE, schedule the exp first -- a finish
                    # op ahead of exp in the FIFO stalls the PE's scores.
                    ctx = tc.high_priority()
                    ctx.__enter__()
                    e = epool.tile([128, 9 * 128], BF16, tag="exp")
                    ss = sing[:, (h % 3) * 128:(h % 3 + 1) * 128]
                    if h < ACT_ONLY:
                        nc.scalar.activation(
                            out=e[:, 0:1024], in_=s_ps[:, 0:1024],
                            func=mybir.ActivationFunctionType.Exp,
                            scale=1.0 / C1,
                        )
                        nc.scalar.activation(
                            out=e[:, 1024:1152], in_=ss,
                            func=mybir.ActivationFunctionType.Exp,
                            scale=1.0 / C1,
                        )
                    else:
                        nc.scalar.activation(
                            out=e[:, 0:act_cols], in_=s_ps[:, 0:act_cols],
                            func=mybir.ActivationFunctionType.Exp,
                            scale=1.0 / C1,
                        )
                        nc.vector.tensor_scalar(
                            out=e[:, act_cols:1024].bitcast(I16),
                            in0=s_ps[:, act_cols:1024],
                            scalar1=C2, scalar2=None,
                            op0=mybir.AluOpType.add,
                        )
                        nc.vector.tensor_scalar(
                            out=e[:, 1024:1152].bitcast(I16),
                            in0=ss,
                            scalar1=C2, scalar2=None,
                            op0=mybir.AluOpType.add,
                        )
                    ctx.__exit__(None, None, None)
                    return e

                def pv(h, e, o_ps):
                    half = h % 2
                    for tt in range(9):
                        t = half * 9 + tt
                        slot = _SLOT[tt]
                        nc.tensor.matmul(
                            o_ps[:, 0:129],
                            e[:, slot * 128:(slot + 1) * 128],
                            vaug[:, t, 0:129],
                            start=(t == 0), stop=(t == KT - 1),
                        )

                def finish(q, o_ps):
                    # ACT (which trails the PV by design) evacuates o to
                    # SBUF so no DVE op ever waits on PV completion --
                    # a PV-dependent op at the DVE FIFO head would starve
                    # the next half's exp and stall the PE.
                    osb = ob1pool.tile([128, 129], F32, tag="osb")
                    nc.scalar.activation(
                        out=osb[:, :], in_=o_ps[:, 0:129],
                        func=mybir.ActivationFunctionType.Copy,
                    )
                    # normalize on DVE (reads SBUF only, so it never waits
                    # on PV at the DVE FIFO head); residual add on GpSimd.
                    recip = sclpool.tile([128, 1], F32, tag="recip")
                    nc.vector.reciprocal(recip[:, :], osb[:, 128:129])
                    ob1 = ob1pool.tile([128, 128], F32, tag="ob1")
                    nc.vector.tensor_scalar(
                        out=ob1[:, :], in0=osb[:, 0:128],
                        scalar1=recip[:, 0:1], scalar2=beta_sb[:, 0:1],
                        op0=mybir.AluOpType.mult, op1=mybir.AluOpType.mult,
                    )
                    ob = outpool.tile([128, 128], F32, tag="ob")
                    nc.gpsimd.tensor_tensor(
                        out=ob[:, :], in0=ob1[:, :], in1=x_nat[:, q, :],
                        op=mybir.AluOpType.add,
                    )
                    dma.dma_start(out_d[q * 128:(q + 1) * 128, :], ob[:, :])

                # finish(q) is emitted two halves AFTER pv(2q+1) so its
                # pv-completion wait never head-of-line-blocks the next
                # half's DVE exp (strict 8-deep FIFO).
                es = []
                o_tiles = {}
                for h in range(NHALF + 4):
                    if h < NHALF:
                        s_ps = scores(h)
                        es.append(exp_half(h, s_ps))
                    hp = h - 2
                    if 0 <= hp < NHALF:
                        if hp % 2 == 0:
                            o_tiles[hp // 2] = opool.tile(
                                [128, 129], F32, tag="o", name="o_ps"
                            )
                        pv(hp, es.pop(0), o_tiles[hp // 2])
                    if h >= 5 and (h - 5) % 2 == 0:
                        q = (h - 5) // 2
                        finish(q, o_tiles.pop(q))

    return nc


_CACHE = {}


def _get_nc(kw_fold):
    key = f"nc{int(kw_fold)}"
    if key not in _CACHE:
        _CACHE[key] = _build_nc(kw_fold)
    return _CACHE[key]


def kernel(query, Wq, bq, Wk, bk, Wv, bv, beta, _trace=False):
    query = np.ascontiguousarray(np.asarray(query, dtype=np.float32))
    Wq = np.asarray(Wq, dtype=np.float32)
    Wk = np.asarray(Wk, dtype=np.float32)
    Wv = np.asarray(Wv, dtype=np.float32)
    bq = np.asarray(bq, dtype=np.float32)
    bk = np.asarray(bk, dtype=np.float32)
    bv = np.asarray(bv, dtype=np.float32)
    beta = np.asarray(beta, dtype=np.float32)

    # replicate q/k weights into PE columns 0-15, 32-47 and 64-79 so
    # their projections land on the row-tiled sub-arrays' SBUF
    # partitions (bands 0/32/64 for the 3-way score group).  Wq/bq are
    # additionally scaled by C1 so the scores arrive pre-scaled for the
    # Schraudolph exp (see module docstring).
    wq2 = np.zeros((C, 80), dtype=np.float32)
    wk2 = np.zeros((C, 80), dtype=np.float32)
    bq2 = np.zeros((80, 1), dtype=np.float32)
    for off in (0, 32, 64):
        wq2[:, off:off + DK] = Wq * C1
        wk2[:, off:off + DK] = Wk
        bq2[off:off + DK, 0] = bq * C1
    # bk never needs to be applied: softmax(q.(k+bk)) == softmax(q.k)
    # (it shifts every logit in a row by the same q.bk constant).

    beta_s = float(beta.ravel()[0])
    # bq == 0 (always, for this harness) enables the kw_fold graph: the
    # q projection folds into the score matmuls via KWT = C1*Wq@k^T.
    kw_fold = not np.any(bq)
    shared = {
        "wv": Wv.astype(ml_dtypes.bfloat16),
        "wk2": wk2.astype(ml_dtypes.bfloat16),
        "beta128": np.full((C, 1), beta_s, dtype=np.float32),
    }
    if kw_fold:
        shared["wqt"] = np.ascontiguousarray(
            (Wq * C1).T.astype(ml_dtypes.bfloat16)
        )
    else:
        shared["wq2"] = wq2.astype(ml_dtypes.bfloat16)
        shared["bq2"] = bq2
    # fold beta*bv into the residual: out = (x + beta*bv) + beta*(o/d)
    bv_fold = (beta_s * bv).astype(np.float32)[None, :]
    in_maps = []
    for b in range(B):
        img = query[b].reshape(N, C)
        m = dict(shared)
        m["x32"] = img + bv_fold
        m["xt"] = np.ascontiguousarray(img.T.astype(ml_dtypes.bfloat16))
        in_maps.append(m)

    nc = _get_nc(kw_fold)
    res = run_bass_kernel_spmd(
        nc, in_maps, core_ids=list(range(B)), trace=_trace
    )
    out = np.stack([res.results[b]["out"] for b in range(B)])
    out = out.reshape(B, H, W, C).astype(np.float32)
    if _trace:
        kernel.last_exec_time_ns = res.exec_time_ns
        kernel.last_profile_json = res.profile_json
    return out

